# revision 3
# baseline (speedup 1.0000x reference)
"""Trainium2 8-core tensor-parallel sparse-attention kernel (Bass/Tile).

Reference (SQ=2048, B=1, H=2048, NH=16, HD=128):
    x = hidden[:,0,:] @ svd_token
    w = qkv_w @ svd_token;  mixed = x @ w.T + qkv_b
    per head h: q,k rotated by svd_qk[h], v by svd_vlin[h]
    scores = qr @ kr.T / sqrt(128) causal-masked, softmax
    ctx = probs @ vr;  tsr[h] = svd_vlin[h].T @ dense_w[h]
    out = ctx @ tsr + dense_b

Key identity (host fold): mixed = X @ (S S^T Q^T) + b, and the per-head
q/k/v rotations are linear, so the WHOLE projection collapses to
    qrot_h = X @ Wq_h + bq_h,  Wq_h = (S S^T Q^T)[:, qcols_h] @ svd_qk[h]
(same for k, v with svd_vlin), all folded in fp32 on the host. tsr is
also fully precomputed host-side. Device work per core drops to:
  Q:   qrotT/krotT/vrot for 2 heads = X-contraction only (6.4 GF)
  att: causal-blocked scores/exp/PV with band-preload masking
  E:   out_block = ctx @ tsr + dense_b after per-head A2A of ctx
No AllGathers of intermediates remain (only warmup + 2 per-head A2A).

Per-core pipeline (TP over heads, 2 heads/core):
  warmup AG first (CC stream init ~55us)
  hT streams on sync queue; wqk/wv/biases on gpsimd queue
  sweep A/B: qrotT,krotT per head  psum[128,512]x8, k-streamed
  sweep C:   vrot both heads, chunk-major psum[128,256], k inner
  tsr (8MB) loads on sync queue after hT's pool releases
  attention per head: two rb-chains interleaved; causal band preloaded
  into PSUM by an identity matmul; raw exp on [128,1024] tiles; P@V +
  ones row-sum; normalize via partition_broadcast + reciprocal
  A2A(h0) hides under h1's attention; stage E splits h0/h1 halves so
  the h0 half of the contraction overlaps A2A(h1); dense_b enters via
  a K=1 f32r matmul preload of the output accumulators.
Host only shards/folds inputs and concatenates the 8 output row-blocks.
"""
import math

import ml_dtypes
import numpy as np

import concourse.bass as bass
import concourse.mybir as mybir
import concourse.bacc as bacc
import concourse.tile as tile
from concourse import bass_utils

N_CORES = 8
SQ = 2048
H = 2048
NH = 16
HD = 128
HPC = NH // N_CORES          # heads per core = 2
SEQB = SQ // N_CORES         # seq block per core = 256
KT = H // 128                # 128-tiles over hidden = 16
F32 = mybir.dt.float32
F32R = mybir.dt.float32r
BF16 = mybir.dt.bfloat16
FP16 = mybir.dt.float16
SCALE = 1.0 / math.sqrt(HD)


def r(ap):
    return ap.bitcast(F32R)


def build(causal=True):
    nc = bacc.Bacc("TRN2", target_bir_lowering=False, debug=False,
                   num_devices=N_CORES)

    hT = nc.dram_tensor("hT", [H, SQ], FP16, kind="ExternalInput")
    wqk = nc.dram_tensor("wqk", [H, 4 * HD], FP16, kind="ExternalInput")
    wv = nc.dram_tensor("wv", [H, 2 * HD], FP16, kind="ExternalInput")
    bqk = nc.dram_tensor("bqk", [128, 4], F32, kind="ExternalInput")
    bvB = nc.dram_tensor("bvB", [128, 2 * HD], F32, kind="ExternalInput")
    tsr = nc.dram_tensor("tsr", [H, H], FP16, kind="ExternalInput")
    dbB = nc.dram_tensor("dbB", [1, H], F32, kind="ExternalInput")
    out = nc.dram_tensor("out", [SEQB, H], F32, kind="ExternalOutput")

    ones_dram = nc.inline_tensor(np.ones((128, 128), np.float32), name="ones_c")
    onesb_dram = nc.inline_tensor(np.ones((128, 128), ml_dtypes.bfloat16),
                                  name="onesb_c")
    idh_dram = nc.inline_tensor(np.eye(128, dtype=np.float16), name="idh_c")
    # additive causal mask band (-30000 above diagonal), preloaded
    # into PSUM via an identity matmul so masking never leaves the PE
    tbh_np = np.where(
        np.arange(128)[:, None] > np.arange(896)[None, :] - 384, -30000.0, 0.0
    ).astype(np.float16)
    tbh_dram = nc.inline_tensor(tbh_np, name="tbh_c")

    rg = [list(range(N_CORES))]

    with tile.TileContext(nc) as tc:
        with (
            nc.allow_low_precision(reason="f32r/bf16 for full-rate PE"),
            tc.tile_pool(name="pers", bufs=1) as pers,
            tc.tile_pool(name="dram", bufs=1, space="DRAM") as dram,
        ):
            # ---- warmup collective ASAP (CC stream init ~55us) ----
            warm_in = dram.tile([128, 128], F32)
            warm_out = dram.tile([N_CORES * 128, 128], F32,
                                 addr_space="Shared")
            nc.sync.dma_start(warm_in[:], ones_dram[:])
            nc.gpsimd.collective_compute(
                "AllGather", mybir.AluOpType.bypass, replica_groups=rg,
                ins=[warm_in[:].opt()], outs=[warm_out[:].opt()])

            # ---- persistent constants (gpsimd queue) ----
            ones_sb = pers.tile([128, 128], F32)
            onesb_sb = pers.tile([128, 128], BF16)
            tbh_sb = pers.tile([128, 896], FP16)
            idb_sb = pers.tile([128, 128], FP16)
            nc.gpsimd.dma_start(idb_sb[:], idh_dram[:])
            nc.gpsimd.dma_start(r(ones_sb[:]), r(ones_dram[:]))
            nc.gpsimd.dma_start(onesb_sb[:], onesb_dram[:])
            nc.gpsimd.dma_start(tbh_sb[:], tbh_dram[:])
            bqk_sb = pers.tile([128, 4], F32)
            nc.gpsimd.dma_start(bqk_sb[:], bqk[:])
            bvB_sb = pers.tile([128, 2 * HD], F32)
            nc.gpsimd.dma_start(r(bvB_sb[:]), r(bvB[:]))
            db_sb = pers.tile([1, H], F32)
            nc.gpsimd.dma_start(r(db_sb[:]), r(dbB[:]))

            ctx_ins = [dram.tile([N_CORES, HD, SEQB], FP16,
                                 name=f"ctxin{hl}") for hl in range(HPC)]
            ctx_as = [dram.tile([N_CORES, HD, SEQB], FP16,
                                name=f"ctxa{hl}") for hl in range(HPC)]

            # ---- stage Q inputs: hT on sync queue, weights on gpsimd ----
            sQ = tc.alloc_tile_pool(name="sQ", bufs=1)
            wqk_sb = sQ.tile([128, KT * 4 * HD], FP16, name="wqk_sb")
            wv_sb = sQ.tile([128, KT * 2 * HD], FP16, name="wv_sb")
            hT_sb = sQ.tile([128, KT * SQ], FP16, name="hT_sb")
            for k in range(KT):
                nc.gpsimd.dma_start(wqk_sb[:, k * 512:(k + 1) * 512],
                                    wqk[k * 128:(k + 1) * 128, :])
            for k in range(KT):
                nc.gpsimd.dma_start(wv_sb[:, k * 256:(k + 1) * 256],
                                    wv[k * 128:(k + 1) * 128, :])
            for k2 in range(KT // 2):
                nc.sync.dma_start(
                    hT_sb[:].rearrange(
                        "p (k s) -> p k s", k=KT)[:, k2 * 2:(k2 + 1) * 2],
                    hT.rearrange("(k p) s -> p k s",
                                 p=128)[:, k2 * 2:(k2 + 1) * 2])

            # attention-lifetime SBUF tiles
            sAtt = tc.alloc_tile_pool(name="sAtt", bufs=1)
            qrotTs = [sAtt.tile([128, SQ], FP16, name=f"qrotT{hl}")
                      for hl in range(HPC)]
            krotTs = [sAtt.tile([128, SQ], FP16, name=f"krotT{hl}")
                      for hl in range(HPC)]
            vrots = [sAtt.tile([128, SQ], BF16, name=f"vrot{hl}")
                     for hl in range(HPC)]
            ctxTs = [sAtt.tile([128, SQ], FP16, name=f"ctxT{hl}")
                     for hl in range(HPC)]

            # ---- sweeps A/B: qrotT/krotT per head, k-streamed ----
            with tc.tile_pool(name="pQ", bufs=8, space="PSUM") as pQ:
                for hl in range(HPC):
                    psq = [pQ.tile([128, 512], F32, tag="acc",
                                   name=f"psq{hl}_{sc}", bufs=8)
                           for sc in range(4)]
                    psk = [pQ.tile([128, 512], F32, tag="acc",
                                   name=f"psk{hl}_{sc}", bufs=8)
                           for sc in range(4)]
                    for k in range(KT):
                        for sc in range(4):
                            rhs = hT_sb[:, k * SQ + sc * 512:
                                        k * SQ + (sc + 1) * 512]
                            nc.tensor.matmul(
                                psq[sc][:],
                                wqk_sb[:, k * 512 + hl * 256:
                                       k * 512 + hl * 256 + 128],
                                rhs, start=(k == 0), stop=(k == KT - 1))
                            nc.tensor.matmul(
                                psk[sc][:],
                                wqk_sb[:, k * 512 + hl * 256 + 128:
                                       k * 512 + hl * 256 + 256],
                                rhs, start=(k == 0), stop=(k == KT - 1))
                    for sc in range(4):
                        nc.vector.tensor_scalar_add(
                            qrotTs[hl][:, sc * 512:(sc + 1) * 512],
                            psq[sc][:], bqk_sb[:, 2 * hl:2 * hl + 1])
                        nc.vector.tensor_scalar_add(
                            krotTs[hl][:, sc * 512:(sc + 1) * 512],
                            psk[sc][:], bqk_sb[:, 2 * hl + 1:2 * hl + 2])

            # ---- sweep C: vrot both heads, chunk-major ----
            with tc.tile_pool(name="pV", bufs=4, space="PSUM") as pV:
                for cp in range(8):
                    vps = [pV.tile([128, 256], F32, tag="vp",
                                   name=f"vp{cp}_{i}", bufs=4)
                           for i in range(2)]
                    for k in range(KT):
                        for i in range(2):
                            c = cp * 2 + i
                            nc.tensor.matmul(
                                vps[i][:],
                                hT_sb[:, k * SQ + c * 128:
                                      k * SQ + (c + 1) * 128],
                                wv_sb[:, k * 256:(k + 1) * 256],
                                start=(k == 0), stop=(k == KT - 1))
                    for i in range(2):
                        c = cp * 2 + i
                        for hl in range(HPC):
                            nc.vector.tensor_tensor(
                                vrots[hl][:, c * 128:(c + 1) * 128],
                                vps[i][:, hl * 128:(hl + 1) * 128],
                                bvB_sb[:, hl * 128:(hl + 1) * 128],
                                mybir.AluOpType.add)

            # hT/wqk/wv dead -> release, then stream tsr into the space
            sQ.release()
            sT = tc.alloc_tile_pool(name="sT", bufs=1)
            tsrb_sb = sT.tile([128, KT * H], FP16, name="tsrb_sb")
            for kt in range(KT):
                nc.sync.dma_start(tsrb_sb[:, kt * H:(kt + 1) * H],
                                  tsr[kt * 128:(kt + 1) * 128, :])

            # ---- attention per head; A2A(h0) hides under h1 ----
            with (
                tc.tile_pool(name="sD", bufs=2) as sD,
                tc.tile_pool(name="pC", bufs=2, space="PSUM") as pC,
            ):
                for hl in range(HPC):
                    qrotT, krotT = qrotTs[hl], krotTs[hl]
                    vrot, ctxT_sb = vrots[hl], ctxTs[hl]

                    # two independent rb-chains interleaved per pair to
                    # keep the PE dense (p-state) within one head
                    for rbp in range(2):
                        rbs = [2 * rbp, 2 * rbp + 1]
                        ncbs = {rb: (4 * (rb + 1) if causal else KT)
                                for rb in rbs}
                        ctps = {rb: pC.tile([128, 512], F32, tag="ctp",
                                            name=f"ctp{hl}_{rb}")
                                for rb in rbs}
                        lps = {rb: pC.tile([1, 512], F32, tag="lp", bufs=2,
                                           name=f"lp{hl}_{rb}")
                               for rb in rbs}
                        maxcp = max(ncbs[rb] // 2 for rb in rbs)
                        for cp in range(maxcp):
                            for rb in rbs:
                                ncb = ncbs[rb]
                                if cp >= ncb // 2:
                                    continue
                                sp = pC.tile([128, 1024], F32, tag="sp",
                                             name=f"sp{hl}_{rb}_{cp}")
                                pT = sD.tile([128, 1024], BF16, tag="pT",
                                             bufs=6,
                                             name=f"pT{hl}_{rb}_{cp}")
                                for ch in range(2):
                                    cb = cp * 2 + ch
                                    sph = sp[:, ch * 512:(ch + 1) * 512]
                                    masked = causal and cb >= 4 * rb
                                    if masked:
                                        o = 384 - (cb * 128 - rb * 512)
                                        nc.tensor.matmul(
                                            sph, idb_sb[:],
                                            tbh_sb[:, o:o + 512],
                                            start=True, stop=False)
                                    nc.tensor.matmul(
                                        sph,
                                        krotT[:, cb * 128:(cb + 1) * 128],
                                        qrotT[:, rb * 512:(rb + 1) * 512],
                                        start=not masked, stop=True)
                                nc.scalar.activation(
                                    pT[:], sp[:],
                                    mybir.ActivationFunctionType.Exp)
                                for ch in range(2):
                                    cb = cp * 2 + ch
                                    nc.tensor.matmul(
                                        ctps[rb][:],
                                        vrot[:, cb * 128:(cb + 1) * 128],
                                        pT[:, ch * 512:(ch + 1) * 512],
                                        start=(cb == 0),
                                        stop=(cb == ncb - 1))
                                for ch in range(2):
                                    cb = cp * 2 + ch
                                    nc.tensor.matmul(
                                        lps[rb][:], onesb_sb[:, 0:1],
                                        pT[:, ch * 512:(ch + 1) * 512],
                                        start=(cb == 0),
                                        stop=(cb == ncb - 1))
                        for rb in rbs:
                            # normalize: lp -> SBUF, partition-broadcast,
                            # fast reciprocal (128 lanes), mult
                            lsb = sD.tile([1, 512], F32, tag="lsb",
                                          name=f"lsb{hl}_{rb}")
                            nc.scalar.copy(lsb[:], lps[rb][:])
                            lball = sD.tile([128, 512], F32, tag="lball",
                                            name=f"lball{hl}_{rb}")
                            nc.gpsimd.partition_broadcast(lball[:], lsb[:])
                            linvb = sD.tile([128, 512], F32, tag="lb",
                                            name=f"linvb{hl}_{rb}")
                            nc.vector.reciprocal_approx_fast(linvb[:],
                                                             lball[:])
                            nc.vector.tensor_tensor(
                                ctxT_sb[:, rb * 512:(rb + 1) * 512],
                                ctps[rb][:],
                                linvb[:], mybir.AluOpType.mult)
                            # stage ctx columns (dest cores 2rb, 2rb+1)
                            nc.sync.dma_start(
                                ctx_ins[hl].rearrange(
                                    "b p s -> p b s")[:, 2 * rb:2 * rb + 2],
                                ctxT_sb[:, rb * 512:(rb + 1) * 512]
                                .rearrange("p (b s) -> p b s", b=2))
                    nc.gpsimd.collective_compute(
                        "AllToAll", mybir.AluOpType.bypass,
                        replica_groups=rg,
                        ins=[ctx_ins[hl][:].opt()],
                        outs=[ctx_as[hl][:].opt()])

            # ---- stage E: out = ctx_myblock @ tsr + dense_b, split so
            #      the h0 half of the contraction overlaps A2A(h1) ----
            with (
                tc.tile_pool(name="sE", bufs=2) as sE,
                tc.tile_pool(name="pE", bufs=8, space="PSUM") as pE,
            ):
                ctxa_sb = sE.tile([128, KT * SEQB], FP16, tag="ctxa", bufs=1)
                ops = [[pE.tile([128, 512], F32, tag="op", bufs=8,
                                name=f"op{mt}_{n}") for n in range(4)]
                       for mt in range(2)]
                # dense_b enters the accumulators via a K=1 f32r matmul
                for mt in range(2):
                    for n in range(4):
                        nc.tensor.matmul(
                            ops[mt][n][:], r(ones_sb[0:1, :]),
                            r(db_sb[:, n * 512:(n + 1) * 512]),
                            start=True, stop=False)
                for hl in range(HPC):
                    nc.sync.dma_start(
                        ctxa_sb[:].rearrange("p (b h s) -> h p b s",
                                             b=N_CORES, h=HPC)[hl],
                        ctx_as[hl][:].rearrange("b p s -> p b s"))
                    for mt in range(2):
                        for b in range(N_CORES):
                            kt = b * HPC + hl
                            for n in range(4):
                                nc.tensor.matmul(
                                    ops[mt][n][:],
                                    ctxa_sb[:, kt * SEQB + mt * 128:
                                            kt * SEQB + (mt + 1) * 128],
                                    tsrb_sb[:, kt * H + n * 512:
                                            kt * H + (n + 1) * 512],
                                    start=False,
                                    stop=(hl == HPC - 1 and
                                          b == N_CORES - 1))
                for mt in range(2):
                    for n in range(4):
                        os_ = sE.tile([128, 512], F32, tag="os")
                        if (mt * 4 + n) % 2:
                            nc.vector.tensor_copy(os_[:], ops[mt][n][:])
                        else:
                            nc.scalar.copy(os_[:], ops[mt][n][:])
                        nc.sync.dma_start(
                            out[mt * 128:(mt + 1) * 128,
                                n * 512:(n + 1) * 512],
                            os_[:])
            sT.release()
            sAtt.release()
    nc.compile()
    return nc


_CAUSAL_MASK = None


def _is_causal(mask):
    global _CAUSAL_MASK
    m = np.asarray(mask).reshape(SQ, SQ)
    if _CAUSAL_MASK is None:
        _CAUSAL_MASK = np.triu(np.ones((SQ, SQ), dtype=bool), k=1)
    return np.array_equal(m, _CAUSAL_MASK)


def make_in_maps(inputs):
    hidden_states = np.asarray(inputs["hidden_states"], np.float32)
    qkv_w = np.asarray(inputs["qkv_w"], np.float32)
    qkv_b = np.asarray(inputs["qkv_b"], np.float32)
    svd_token = np.ascontiguousarray(np.asarray(inputs["svd_token"],
                                                np.float32))
    svd_qk = np.asarray(inputs["svd_qk"], np.float32)
    svd_vlin = np.asarray(inputs["svd_vlin"], np.float32)
    dense_w = np.asarray(inputs["dense_w"], np.float32)
    dense_b = np.asarray(inputs["dense_b"], np.float32)

    # host fold: mixed = X (S S^T Q^T) + b; then per-head q/k/v rotations
    # and the softmax scale (split sqrt into q and k) fold into W/b too.
    sq_scale = math.sqrt(SCALE)
    G = svd_token @ svd_token.T
    Wmix = G @ qkv_w.T                                    # [H, 3H]
    Wh = Wmix.reshape(H, NH, 3 * HD)
    bh = qkv_b.reshape(NH, 3 * HD)
    Wq = np.einsum("xhd,hde->xhe", Wh[:, :, :HD], svd_qk,
                   optimize=True) * sq_scale
    Wk = np.einsum("xhd,hde->xhe", Wh[:, :, HD:2 * HD], svd_qk,
                   optimize=True) * sq_scale
    Wv = np.einsum("xhd,hde->xhe", Wh[:, :, 2 * HD:], svd_vlin,
                   optimize=True)
    bq = np.einsum("hd,hde->he", bh[:, :HD], svd_qk, optimize=True) * sq_scale
    bk = np.einsum("hd,hde->he", bh[:, HD:2 * HD], svd_qk,
                   optimize=True) * sq_scale
    bv = np.einsum("hd,hde->he", bh[:, 2 * HD:], svd_vlin, optimize=True)

    tsr = np.matmul(svd_vlin.transpose(0, 2, 1), dense_w).reshape(H, H)
    tsr_b = np.ascontiguousarray(tsr).astype(np.float16)
    hTf = np.ascontiguousarray(
        hidden_states[:, 0, :].T).astype(np.float16)      # [H, SQ]
    dbB = np.ascontiguousarray(dense_b.reshape(1, H))

    in_maps = []
    for c in range(N_CORES):
        h0 = c * HPC
        wqk_c = np.empty((H, 4 * HD), np.float32)
        wqk_c[:, 0:128] = Wq[:, h0]
        wqk_c[:, 128:256] = Wk[:, h0]
        wqk_c[:, 256:384] = Wq[:, h0 + 1]
        wqk_c[:, 384:512] = Wk[:, h0 + 1]
        wv_c = np.concatenate([Wv[:, h0], Wv[:, h0 + 1]], axis=1)
        bqk_c = np.stack([bq[h0], bk[h0], bq[h0 + 1], bk[h0 + 1]], axis=1)
        bv_c = np.concatenate([bv[h0], bv[h0 + 1]])
        in_maps.append({
            "hT": hTf,
            "wqk": wqk_c.astype(np.float16),
            "wv": wv_c.astype(np.float16),
            "bqk": np.ascontiguousarray(bqk_c, dtype=np.float32),
            "bvB": np.ascontiguousarray(
                np.broadcast_to(bv_c[None, :], (128, 2 * HD)),
                dtype=np.float32),
            "tsr": tsr_b,
            "dbB": dbB,
        })
    return in_maps


def kernel(hidden_states, attention_mask, qkv_w, qkv_b, svd_token,
           svd_qk, svd_vlin, dense_w, dense_b):
    causal = _is_causal(attention_mask)
    if not causal:
        assert not np.asarray(attention_mask).any(), \
            "kernel supports causal or empty attention_mask"

    nc = build(causal=causal)
    in_maps = make_in_maps({
        "hidden_states": hidden_states, "qkv_w": qkv_w, "qkv_b": qkv_b,
        "svd_token": svd_token, "svd_qk": svd_qk, "svd_vlin": svd_vlin,
        "dense_w": dense_w, "dense_b": dense_b,
    })
    res = bass_utils.run_bass_kernel_spmd(
        nc, in_maps, core_ids=list(range(N_CORES)), trace=False)
    full = np.concatenate([res.results[c]["out"] for c in range(N_CORES)],
                          axis=0)
    return full.reshape(SQ, 1, H)


# revision 4
# speedup vs baseline: 1.4399x; 1.4399x over previous
"""Trainium2 8-core tensor-parallel sparse-attention kernel (Bass/Tile).

Reference (SQ=2048, B=1, H=2048, NH=16, HD=128):
    x = hidden[:,0,:] @ svd_token
    w = qkv_w @ svd_token;  mixed = x @ w.T + qkv_b
    per head h: q,k rotated by svd_qk[h], v by svd_vlin[h]
    scores = qr @ kr.T / sqrt(128) causal-masked, softmax
    ctx = probs @ vr;  tsr[h] = svd_vlin[h].T @ dense_w[h]
    out = ctx @ tsr + dense_b

Key identity (host fold): mixed = X @ (S S^T Q^T) + b, and the per-head
q/k/v rotations are linear, so the WHOLE projection collapses to
    qrot_h = X @ Wq_h + bq_h,  Wq_h = (S S^T Q^T)[:, qcols_h] @ svd_qk[h]
(same for k, v with svd_vlin), all folded in fp32 on the host. tsr is
also fully precomputed host-side. Device work per core drops to:
  Q:   qrotT/krotT/vrot for 2 heads = X-contraction only (6.4 GF)
  att: causal-blocked scores/exp/PV with band-preload masking
  E:   out_block = ctx @ tsr + dense_b after per-head A2A of ctx
No AllGathers of intermediates remain (only warmup + 2 per-head A2A).

Per-core pipeline (TP over heads, 2 heads/core):
  warmup AG first (CC stream init ~55us)
  hT streams on sync queue; wqk/wv/biases on gpsimd queue
  sweep A/B: qrotT,krotT per head  psum[128,512]x8, k-streamed
  sweep C:   vrot both heads, chunk-major psum[128,256], k inner
  tsr (8MB) loads on sync queue after hT's pool releases
  attention per head: two rb-chains interleaved; causal band preloaded
  into PSUM by an identity matmul; raw exp on [128,1024] tiles; P@V +
  ones row-sum; normalize via partition_broadcast + reciprocal
  A2A(h0) hides under h1's attention; stage E splits h0/h1 halves so
  the h0 half of the contraction overlaps A2A(h1); dense_b enters via
  a K=1 f32r matmul preload of the output accumulators.
Host only shards/folds inputs and concatenates the 8 output row-blocks.
"""
import math

import ml_dtypes
import numpy as np

import concourse.bass as bass
import concourse.mybir as mybir
import concourse.bacc as bacc
import concourse.tile as tile
from concourse import bass_utils

N_CORES = 8
SQ = 2048
H = 2048
NH = 16
HD = 128
HPC = NH // N_CORES          # heads per core = 2
SEQB = SQ // N_CORES         # seq block per core = 256
KT = H // 128                # 128-tiles over hidden = 16
F32 = mybir.dt.float32
F32R = mybir.dt.float32r
BF16 = mybir.dt.bfloat16
FP16 = mybir.dt.float16
SCALE = 1.0 / math.sqrt(HD)


def r(ap):
    return ap.bitcast(F32R)


def build(causal=True):
    nc = bacc.Bacc("TRN2", target_bir_lowering=False, debug=False,
                   num_devices=N_CORES)

    hT = nc.dram_tensor("hT", [H, SQ], FP16, kind="ExternalInput")
    wqk = nc.dram_tensor("wqk", [H, 4 * HD], FP16, kind="ExternalInput")
    wv = nc.dram_tensor("wv", [H, 2 * HD], FP16, kind="ExternalInput")
    bqk = nc.dram_tensor("bqk", [128, 4], F32, kind="ExternalInput")
    bvB = nc.dram_tensor("bvB", [128, 2 * HD], F32, kind="ExternalInput")
    tsr = nc.dram_tensor("tsr", [H, H], FP16, kind="ExternalInput")
    dbB = nc.dram_tensor("dbB", [1, H], F32, kind="ExternalInput")
    out = nc.dram_tensor("out", [SEQB, H], F32, kind="ExternalOutput")

    ones_dram = nc.inline_tensor(np.ones((128, 128), np.float32), name="ones_c")
    onesb_dram = nc.inline_tensor(np.ones((128, 128), ml_dtypes.bfloat16),
                                  name="onesb_c")
    idh_dram = nc.inline_tensor(np.eye(128, dtype=np.float16), name="idh_c")
    # additive causal mask band (-30000 above diagonal), preloaded
    # into PSUM via an identity matmul so masking never leaves the PE
    tbh_np = np.where(
        np.arange(128)[:, None] > np.arange(896)[None, :] - 384, -30000.0, 0.0
    ).astype(np.float16)
    tbh_dram = nc.inline_tensor(tbh_np, name="tbh_c")

    rg = [list(range(N_CORES))]

    with tile.TileContext(nc) as tc:
        with (
            nc.allow_low_precision(reason="f32r/bf16 for full-rate PE"),
            tc.tile_pool(name="pers", bufs=1) as pers,
            tc.tile_pool(name="dram", bufs=1, space="DRAM") as dram,
        ):
            # ---- warmup collective ASAP (CC stream init ~55us) ----
            warm_in = dram.tile([128, 128], F32)
            warm_out = dram.tile([N_CORES * 128, 128], F32,
                                 addr_space="Shared")
            nc.sync.dma_start(warm_in[:], ones_dram[:])
            nc.gpsimd.collective_compute(
                "AllGather", mybir.AluOpType.bypass, replica_groups=rg,
                ins=[warm_in[:].opt()], outs=[warm_out[:].opt()])

            # ---- persistent constants (gpsimd queue) ----
            ones_sb = pers.tile([128, 128], F32)
            onesb_sb = pers.tile([128, 128], BF16)
            tbh_sb = pers.tile([128, 896], FP16)
            idb_sb = pers.tile([128, 128], FP16)
            nc.gpsimd.dma_start(idb_sb[:], idh_dram[:])
            nc.gpsimd.dma_start(r(ones_sb[:]), r(ones_dram[:]))
            nc.gpsimd.dma_start(onesb_sb[:], onesb_dram[:])
            nc.gpsimd.dma_start(tbh_sb[:], tbh_dram[:])
            bqk_sb = pers.tile([128, 4], F32)
            nc.gpsimd.dma_start(bqk_sb[:], bqk[:])
            bvB_sb = pers.tile([128, 2 * HD], F32)
            nc.gpsimd.dma_start(r(bvB_sb[:]), r(bvB[:]))
            db_sb = pers.tile([1, H], F32)
            nc.gpsimd.dma_start(r(db_sb[:]), r(dbB[:]))

            ctx_ins = [dram.tile([N_CORES, HD, SEQB], FP16,
                                 name=f"ctxin{hl}") for hl in range(HPC)]
            ctx_as = [dram.tile([N_CORES, HD, SEQB], FP16,
                                name=f"ctxa{hl}") for hl in range(HPC)]

            # ---- stage Q inputs: hT on sync queue, weights on gpsimd ----
            sQ = tc.alloc_tile_pool(name="sQ", bufs=1)
            wqk_sb = sQ.tile([128, KT * 4 * HD], FP16, name="wqk_sb")
            wv_sb = sQ.tile([128, KT * 2 * HD], FP16, name="wv_sb")
            hT_sb = sQ.tile([128, KT * SQ], FP16, name="hT_sb")
            for k in range(KT):
                nc.gpsimd.dma_start(wqk_sb[:, k * 512:(k + 1) * 512],
                                    wqk[k * 128:(k + 1) * 128, :])
            for k in range(KT):
                nc.gpsimd.dma_start(wv_sb[:, k * 256:(k + 1) * 256],
                                    wv[k * 128:(k + 1) * 128, :])
            for k2 in range(KT // 2):
                nc.sync.dma_start(
                    hT_sb[:].rearrange(
                        "p (k s) -> p k s", k=KT)[:, k2 * 2:(k2 + 1) * 2],
                    hT.rearrange("(k p) s -> p k s",
                                 p=128)[:, k2 * 2:(k2 + 1) * 2])

            # attention-lifetime SBUF tiles (right side: outlives sQ,
            # which must release in LIFO order on the left stack)
            sAtt = tc.alloc_tile_pool(name="sAtt", bufs=1, side="right")
            qrotTs = [sAtt.tile([128, SQ], FP16, name=f"qrotT{hl}")
                      for hl in range(HPC)]
            krotTs = [sAtt.tile([128, SQ], FP16, name=f"krotT{hl}")
                      for hl in range(HPC)]
            vrots = [sAtt.tile([128, SQ], BF16, name=f"vrot{hl}")
                     for hl in range(HPC)]
            ctxTs = [sAtt.tile([128, SQ], FP16, name=f"ctxT{hl}")
                     for hl in range(HPC)]

            # ---- sweeps A/B: qrotT/krotT per head, k-streamed ----
            with tc.tile_pool(name="pQ", bufs=8, space="PSUM") as pQ:
                for hl in range(HPC):
                    psq = [pQ.tile([128, 512], F32, tag="acc",
                                   name=f"psq{hl}_{sc}", bufs=8)
                           for sc in range(4)]
                    psk = [pQ.tile([128, 512], F32, tag="acc",
                                   name=f"psk{hl}_{sc}", bufs=8)
                           for sc in range(4)]
                    for k in range(KT):
                        for sc in range(4):
                            rhs = hT_sb[:, k * SQ + sc * 512:
                                        k * SQ + (sc + 1) * 512]
                            nc.tensor.matmul(
                                psq[sc][:],
                                wqk_sb[:, k * 512 + hl * 256:
                                       k * 512 + hl * 256 + 128],
                                rhs, start=(k == 0), stop=(k == KT - 1))
                            nc.tensor.matmul(
                                psk[sc][:],
                                wqk_sb[:, k * 512 + hl * 256 + 128:
                                       k * 512 + hl * 256 + 256],
                                rhs, start=(k == 0), stop=(k == KT - 1))
                    for sc in range(4):
                        nc.vector.tensor_scalar_add(
                            qrotTs[hl][:, sc * 512:(sc + 1) * 512],
                            psq[sc][:], bqk_sb[:, 2 * hl:2 * hl + 1])
                        nc.vector.tensor_scalar_add(
                            krotTs[hl][:, sc * 512:(sc + 1) * 512],
                            psk[sc][:], bqk_sb[:, 2 * hl + 1:2 * hl + 2])

            # ---- sweep C: vrot both heads, chunk-major ----
            with tc.tile_pool(name="pV", bufs=4, space="PSUM") as pV:
                for cp in range(8):
                    vps = [pV.tile([128, 256], F32, tag="vp",
                                   name=f"vp{cp}_{i}", bufs=4)
                           for i in range(2)]
                    for k in range(KT):
                        for i in range(2):
                            c = cp * 2 + i
                            nc.tensor.matmul(
                                vps[i][:],
                                hT_sb[:, k * SQ + c * 128:
                                      k * SQ + (c + 1) * 128],
                                wv_sb[:, k * 256:(k + 1) * 256],
                                start=(k == 0), stop=(k == KT - 1))
                    for i in range(2):
                        c = cp * 2 + i
                        for hl in range(HPC):
                            nc.vector.tensor_tensor(
                                vrots[hl][:, c * 128:(c + 1) * 128],
                                vps[i][:, hl * 128:(hl + 1) * 128],
                                bvB_sb[:, hl * 128:(hl + 1) * 128],
                                mybir.AluOpType.add)

            # hT/wqk/wv dead -> release, then stream tsr into the space
            sQ.release()
            sT = tc.alloc_tile_pool(name="sT", bufs=1)
            tsrb_sb = sT.tile([128, KT * H], FP16, name="tsrb_sb")
            for kt in range(KT):
                nc.sync.dma_start(tsrb_sb[:, kt * H:(kt + 1) * H],
                                  tsr[kt * 128:(kt + 1) * 128, :])

            # ---- attention per head; A2A(h0) hides under h1 ----
            with (
                tc.tile_pool(name="sD", bufs=2) as sD,
                tc.tile_pool(name="pC", bufs=2, space="PSUM") as pC,
            ):
                for hl in range(HPC):
                    qrotT, krotT = qrotTs[hl], krotTs[hl]
                    vrot, ctxT_sb = vrots[hl], ctxTs[hl]

                    # two independent rb-chains interleaved per pair to
                    # keep the PE dense (p-state) within one head
                    for rbp in range(2):
                        rbs = [2 * rbp, 2 * rbp + 1]
                        ncbs = {rb: (4 * (rb + 1) if causal else KT)
                                for rb in rbs}
                        ctps = {rb: pC.tile([128, 512], F32, tag="ctp",
                                            name=f"ctp{hl}_{rb}")
                                for rb in rbs}
                        lps = {rb: pC.tile([1, 512], F32, tag="lp", bufs=2,
                                           name=f"lp{hl}_{rb}")
                               for rb in rbs}
                        maxcp = max(ncbs[rb] // 2 for rb in rbs)
                        for cp in range(maxcp):
                            for rb in rbs:
                                ncb = ncbs[rb]
                                if cp >= ncb // 2:
                                    continue
                                sp = pC.tile([128, 1024], F32, tag="sp",
                                             name=f"sp{hl}_{rb}_{cp}")
                                pT = sD.tile([128, 1024], BF16, tag="pT",
                                             bufs=6,
                                             name=f"pT{hl}_{rb}_{cp}")
                                for ch in range(2):
                                    cb = cp * 2 + ch
                                    sph = sp[:, ch * 512:(ch + 1) * 512]
                                    masked = causal and cb >= 4 * rb
                                    if masked:
                                        o = 384 - (cb * 128 - rb * 512)
                                        nc.tensor.matmul(
                                            sph, idb_sb[:],
                                            tbh_sb[:, o:o + 512],
                                            start=True, stop=False)
                                    nc.tensor.matmul(
                                        sph,
                                        krotT[:, cb * 128:(cb + 1) * 128],
                                        qrotT[:, rb * 512:(rb + 1) * 512],
                                        start=not masked, stop=True)
                                nc.scalar.activation(
                                    pT[:], sp[:],
                                    mybir.ActivationFunctionType.Exp)
                                for ch in range(2):
                                    cb = cp * 2 + ch
                                    nc.tensor.matmul(
                                        ctps[rb][:],
                                        vrot[:, cb * 128:(cb + 1) * 128],
                                        pT[:, ch * 512:(ch + 1) * 512],
                                        start=(cb == 0),
                                        stop=(cb == ncb - 1))
                                for ch in range(2):
                                    cb = cp * 2 + ch
                                    nc.tensor.matmul(
                                        lps[rb][:], onesb_sb[:, 0:1],
                                        pT[:, ch * 512:(ch + 1) * 512],
                                        start=(cb == 0),
                                        stop=(cb == ncb - 1))
                        for rb in rbs:
                            # normalize: lp -> SBUF, partition-broadcast,
                            # fast reciprocal (128 lanes), mult
                            lsb = sD.tile([1, 512], F32, tag="lsb",
                                          name=f"lsb{hl}_{rb}")
                            nc.scalar.copy(lsb[:], lps[rb][:])
                            lball = sD.tile([128, 512], F32, tag="lball",
                                            name=f"lball{hl}_{rb}")
                            nc.gpsimd.partition_broadcast(lball[:], lsb[:])
                            linvb = sD.tile([128, 512], F32, tag="lb",
                                            name=f"linvb{hl}_{rb}")
                            nc.vector.reciprocal_approx_fast(linvb[:],
                                                             lball[:])
                            nc.vector.tensor_tensor(
                                ctxT_sb[:, rb * 512:(rb + 1) * 512],
                                ctps[rb][:],
                                linvb[:], mybir.AluOpType.mult)
                            # stage ctx columns (dest cores 2rb, 2rb+1)
                            nc.sync.dma_start(
                                ctx_ins[hl].rearrange(
                                    "b p s -> p b s")[:, 2 * rb:2 * rb + 2],
                                ctxT_sb[:, rb * 512:(rb + 1) * 512]
                                .rearrange("p (b s) -> p b s", b=2))
                    nc.gpsimd.collective_compute(
                        "AllToAll", mybir.AluOpType.bypass,
                        replica_groups=rg,
                        ins=[ctx_ins[hl][:].opt()],
                        outs=[ctx_as[hl][:].opt()])

            # ---- stage E: out = ctx_myblock @ tsr + dense_b, split so
            #      the h0 half of the contraction overlaps A2A(h1) ----
            with (
                tc.tile_pool(name="sE", bufs=2) as sE,
                tc.tile_pool(name="pE", bufs=8, space="PSUM") as pE,
            ):
                ctxa_sb = sE.tile([128, KT * SEQB], FP16, tag="ctxa", bufs=1)
                ops = [[pE.tile([128, 512], F32, tag="op", bufs=8,
                                name=f"op{mt}_{n}") for n in range(4)]
                       for mt in range(2)]
                # dense_b enters the accumulators via a K=1 f32r matmul
                for mt in range(2):
                    for n in range(4):
                        nc.tensor.matmul(
                            ops[mt][n][:], r(ones_sb[0:1, :]),
                            r(db_sb[:, n * 512:(n + 1) * 512]),
                            start=True, stop=False)
                for hl in range(HPC):
                    nc.sync.dma_start(
                        ctxa_sb[:].rearrange("p (b h s) -> h p b s",
                                             b=N_CORES, h=HPC)[hl],
                        ctx_as[hl][:].rearrange("b p s -> p b s"))
                    for mt in range(2):
                        for b in range(N_CORES):
                            kt = b * HPC + hl
                            for n in range(4):
                                nc.tensor.matmul(
                                    ops[mt][n][:],
                                    ctxa_sb[:, kt * SEQB + mt * 128:
                                            kt * SEQB + (mt + 1) * 128],
                                    tsrb_sb[:, kt * H + n * 512:
                                            kt * H + (n + 1) * 512],
                                    start=False,
                                    stop=(hl == HPC - 1 and
                                          b == N_CORES - 1))
                for mt in range(2):
                    for n in range(4):
                        os_ = sE.tile([128, 512], F32, tag="os")
                        if (mt * 4 + n) % 2:
                            nc.vector.tensor_copy(os_[:], ops[mt][n][:])
                        else:
                            nc.scalar.copy(os_[:], ops[mt][n][:])
                        nc.sync.dma_start(
                            out[mt * 128:(mt + 1) * 128,
                                n * 512:(n + 1) * 512],
                            os_[:])
            sT.release()
            sAtt.release()
    nc.compile()
    return nc


_CAUSAL_MASK = None


def _is_causal(mask):
    global _CAUSAL_MASK
    m = np.asarray(mask).reshape(SQ, SQ)
    if _CAUSAL_MASK is None:
        _CAUSAL_MASK = np.triu(np.ones((SQ, SQ), dtype=bool), k=1)
    return np.array_equal(m, _CAUSAL_MASK)


def make_in_maps(inputs):
    hidden_states = np.asarray(inputs["hidden_states"], np.float32)
    qkv_w = np.asarray(inputs["qkv_w"], np.float32)
    qkv_b = np.asarray(inputs["qkv_b"], np.float32)
    svd_token = np.ascontiguousarray(np.asarray(inputs["svd_token"],
                                                np.float32))
    svd_qk = np.asarray(inputs["svd_qk"], np.float32)
    svd_vlin = np.asarray(inputs["svd_vlin"], np.float32)
    dense_w = np.asarray(inputs["dense_w"], np.float32)
    dense_b = np.asarray(inputs["dense_b"], np.float32)

    # host fold: mixed = X (S S^T Q^T) + b; then per-head q/k/v rotations
    # and the softmax scale (split sqrt into q and k) fold into W/b too.
    sq_scale = math.sqrt(SCALE)
    G = svd_token @ svd_token.T
    Wmix = G @ qkv_w.T                                    # [H, 3H]
    Wh = Wmix.reshape(H, NH, 3 * HD)
    bh = qkv_b.reshape(NH, 3 * HD)
    Wq = np.einsum("xhd,hde->xhe", Wh[:, :, :HD], svd_qk,
                   optimize=True) * sq_scale
    Wk = np.einsum("xhd,hde->xhe", Wh[:, :, HD:2 * HD], svd_qk,
                   optimize=True) * sq_scale
    Wv = np.einsum("xhd,hde->xhe", Wh[:, :, 2 * HD:], svd_vlin,
                   optimize=True)
    bq = np.einsum("hd,hde->he", bh[:, :HD], svd_qk, optimize=True) * sq_scale
    bk = np.einsum("hd,hde->he", bh[:, HD:2 * HD], svd_qk,
                   optimize=True) * sq_scale
    bv = np.einsum("hd,hde->he", bh[:, 2 * HD:], svd_vlin, optimize=True)

    tsr = np.matmul(svd_vlin.transpose(0, 2, 1), dense_w).reshape(H, H)
    tsr_b = np.ascontiguousarray(tsr).astype(np.float16)
    hTf = np.ascontiguousarray(
        hidden_states[:, 0, :].T).astype(np.float16)      # [H, SQ]
    dbB = np.ascontiguousarray(dense_b.reshape(1, H))

    in_maps = []
    for c in range(N_CORES):
        h0 = c * HPC
        wqk_c = np.empty((H, 4 * HD), np.float32)
        wqk_c[:, 0:128] = Wq[:, h0]
        wqk_c[:, 128:256] = Wk[:, h0]
        wqk_c[:, 256:384] = Wq[:, h0 + 1]
        wqk_c[:, 384:512] = Wk[:, h0 + 1]
        wv_c = np.concatenate([Wv[:, h0], Wv[:, h0 + 1]], axis=1)
        bqk_c = np.stack([bq[h0], bk[h0], bq[h0 + 1], bk[h0 + 1]], axis=1)
        bv_c = np.concatenate([bv[h0], bv[h0 + 1]])
        in_maps.append({
            "hT": hTf,
            "wqk": wqk_c.astype(np.float16),
            "wv": wv_c.astype(np.float16),
            "bqk": np.ascontiguousarray(bqk_c, dtype=np.float32),
            "bvB": np.ascontiguousarray(
                np.broadcast_to(bv_c[None, :], (128, 2 * HD)),
                dtype=np.float32),
            "tsr": tsr_b,
            "dbB": dbB,
        })
    return in_maps


def kernel(hidden_states, attention_mask, qkv_w, qkv_b, svd_token,
           svd_qk, svd_vlin, dense_w, dense_b):
    causal = _is_causal(attention_mask)
    if not causal:
        assert not np.asarray(attention_mask).any(), \
            "kernel supports causal or empty attention_mask"

    nc = build(causal=causal)
    in_maps = make_in_maps({
        "hidden_states": hidden_states, "qkv_w": qkv_w, "qkv_b": qkv_b,
        "svd_token": svd_token, "svd_qk": svd_qk, "svd_vlin": svd_vlin,
        "dense_w": dense_w, "dense_b": dense_b,
    })
    res = bass_utils.run_bass_kernel_spmd(
        nc, in_maps, core_ids=list(range(N_CORES)), trace=False)
    full = np.concatenate([res.results[c]["out"] for c in range(N_CORES)],
                          axis=0)
    return full.reshape(SQ, 1, H)


# revision 8
# speedup vs baseline: 1.4566x; 1.0116x over previous
"""Trainium2 8-core tensor-parallel sparse-attention kernel (Bass/Tile).

Reference (SQ=2048, B=1, H=2048, NH=16, HD=128):
    x = hidden[:,0,:] @ svd_token
    w = qkv_w @ svd_token;  mixed = x @ w.T + qkv_b
    per head h: q,k rotated by svd_qk[h], v by svd_vlin[h]
    scores = qr @ kr.T / sqrt(128) causal-masked, softmax
    ctx = probs @ vr;  tsr[h] = svd_vlin[h].T @ dense_w[h]
    out = ctx @ tsr + dense_b

Key identity (host fold): mixed = X @ (S S^T Q^T) + b, and the per-head
q/k/v rotations are linear, so the WHOLE projection collapses to
    qrot_h = X @ Wq_h + bq_h,  Wq_h = (S S^T Q^T)[:, qcols_h] @ svd_qk[h]
(same for k, v with svd_vlin), all folded in fp32 on the host. tsr is
also fully precomputed host-side. Device work per core drops to:
  Q:   qrotT/krotT/vrot for 2 heads = X-contraction only (6.4 GF)
  att: causal-blocked scores/exp/PV with band-preload masking
  E:   out_block = ctx @ tsr + dense_b after per-head A2A of ctx
No AllGathers of intermediates remain (only warmup + 2 per-head A2A).

Per-core pipeline (TP over heads, 2 heads/core):
  warmup AG first (CC stream init ~55us)
  hT streams on sync queue; wqk/wv/biases on gpsimd queue
  sweep A/B: qrotT,krotT per head  psum[128,512]x8, k-streamed
  sweep C:   vrot both heads, chunk-major psum[128,256], k inner
  tsr (8MB) loads on sync queue after hT's pool releases
  attention per head: two rb-chains interleaved; causal band preloaded
  into PSUM by an identity matmul; raw exp on [128,1024] tiles; P@V +
  ones row-sum; normalize via partition_broadcast + reciprocal
  A2A(h0) hides under h1's attention; stage E splits h0/h1 halves so
  the h0 half of the contraction overlaps A2A(h1); dense_b enters via
  a K=1 f32r matmul preload of the output accumulators.
Host only shards/folds inputs and concatenates the 8 output row-blocks.
"""
import math

import ml_dtypes
import numpy as np

import concourse.bass as bass
import concourse.mybir as mybir
import concourse.bacc as bacc
import concourse.tile as tile
from concourse import bass_utils

N_CORES = 8
SQ = 2048
H = 2048
NH = 16
HD = 128
HPC = NH // N_CORES          # heads per core = 2
SEQB = SQ // N_CORES         # seq block per core = 256
KT = H // 128                # 128-tiles over hidden = 16
F32 = mybir.dt.float32
F32R = mybir.dt.float32r
BF16 = mybir.dt.bfloat16
FP16 = mybir.dt.float16
SCALE = 1.0 / math.sqrt(HD)


def r(ap):
    return ap.bitcast(F32R)


def build(causal=True):
    nc = bacc.Bacc("TRN2", target_bir_lowering=False, debug=False,
                   num_devices=N_CORES)

    hT = nc.dram_tensor("hT", [H, SQ], FP16, kind="ExternalInput")
    wqk = nc.dram_tensor("wqk", [H, 4 * HD], FP16, kind="ExternalInput")
    wv = nc.dram_tensor("wv", [H, 2 * HD], FP16, kind="ExternalInput")
    bqk = nc.dram_tensor("bqk", [128, 4], F32, kind="ExternalInput")
    bvB = nc.dram_tensor("bvB", [128, 2 * HD], F32, kind="ExternalInput")
    tsr = nc.dram_tensor("tsr", [H, H], FP16, kind="ExternalInput")
    dbB = nc.dram_tensor("dbB", [1, H], F32, kind="ExternalInput")
    out = nc.dram_tensor("out", [SEQB, H], F32, kind="ExternalOutput")

    ones_dram = nc.inline_tensor(np.ones((128, 128), np.float32), name="ones_c")
    onesb_dram = nc.inline_tensor(np.ones((128, 128), ml_dtypes.bfloat16),
                                  name="onesb_c")
    idh_dram = nc.inline_tensor(np.eye(128, dtype=np.float16), name="idh_c")
    # additive causal mask band (-30000 above diagonal), preloaded
    # into PSUM via an identity matmul so masking never leaves the PE
    tbh_np = np.where(
        np.arange(128)[:, None] > np.arange(896)[None, :] - 384, -30000.0, 0.0
    ).astype(np.float16)
    tbh_dram = nc.inline_tensor(tbh_np, name="tbh_c")

    rg = [list(range(N_CORES))]

    with tile.TileContext(nc) as tc:
        with (
            nc.allow_low_precision(reason="f32r/bf16 for full-rate PE"),
            tc.tile_pool(name="pers", bufs=1) as pers,
            tc.tile_pool(name="dram", bufs=1, space="DRAM") as dram,
        ):
            # ---- warmup collective ASAP (CC stream init ~55us) ----
            warm_in = dram.tile([128, 128], F32)
            warm_out = dram.tile([N_CORES * 128, 128], F32,
                                 addr_space="Shared")
            nc.sync.dma_start(warm_in[:], ones_dram[:])
            nc.gpsimd.collective_compute(
                "AllGather", mybir.AluOpType.bypass, replica_groups=rg,
                ins=[warm_in[:].opt()], outs=[warm_out[:].opt()])

            # ---- persistent constants (gpsimd queue) ----
            ones_sb = pers.tile([128, 128], F32)
            onesb_sb = pers.tile([128, 128], BF16)
            tbh_sb = pers.tile([128, 896], FP16)
            idb_sb = pers.tile([128, 128], FP16)
            nc.gpsimd.dma_start(idb_sb[:], idh_dram[:])
            nc.gpsimd.dma_start(r(ones_sb[:]), r(ones_dram[:]))
            nc.gpsimd.dma_start(onesb_sb[:], onesb_dram[:])
            nc.gpsimd.dma_start(tbh_sb[:], tbh_dram[:])
            bqk_sb = pers.tile([128, 4], F32)
            nc.gpsimd.dma_start(bqk_sb[:], bqk[:])
            bvB_sb = pers.tile([128, 2 * HD], F32)
            nc.gpsimd.dma_start(r(bvB_sb[:]), r(bvB[:]))
            db_sb = pers.tile([1, H], F32)
            nc.gpsimd.dma_start(r(db_sb[:]), r(dbB[:]))

            ctx_ins = [dram.tile([N_CORES, HD, SEQB], FP16,
                                 name=f"ctxin{hl}") for hl in range(HPC)]
            ctx_as = [dram.tile([N_CORES, HD, SEQB], FP16,
                                name=f"ctxa{hl}") for hl in range(HPC)]

            # ---- stage Q inputs, all on the sync queue in consumption
            #      order: wqk (one shot), hT pairs, wv (needed at C) ----
            sQ = tc.alloc_tile_pool(name="sQ", bufs=1)
            wqk_sb = sQ.tile([128, KT * 4 * HD], FP16, name="wqk_sb")
            wv_sb = sQ.tile([128, KT * 2 * HD], FP16, name="wv_sb")
            hT_sb = sQ.tile([128, KT * SQ], FP16, name="hT_sb")
            nc.sync.dma_start(
                wqk_sb[:].rearrange("p (k c) -> p k c", k=KT),
                wqk.rearrange("(k p) c -> p k c", p=128))
            for k2 in range(KT // 2):
                nc.sync.dma_start(
                    hT_sb[:].rearrange(
                        "p (k s) -> p k s", k=KT)[:, k2 * 2:(k2 + 1) * 2],
                    hT.rearrange("(k p) s -> p k s",
                                 p=128)[:, k2 * 2:(k2 + 1) * 2])
            nc.sync.dma_start(
                wv_sb[:].rearrange("p (k c) -> p k c", k=KT),
                wv.rearrange("(k p) c -> p k c", p=128))

            # attention-lifetime SBUF tiles (right side: outlives sQ,
            # which must release in LIFO order on the left stack)
            sAtt = tc.alloc_tile_pool(name="sAtt", bufs=1, side="right")
            qrotTs = [sAtt.tile([128, SQ], FP16, name=f"qrotT{hl}")
                      for hl in range(HPC)]
            krotTs = [sAtt.tile([128, SQ], FP16, name=f"krotT{hl}")
                      for hl in range(HPC)]
            vrots = [sAtt.tile([128, SQ], BF16, name=f"vrot{hl}")
                     for hl in range(HPC)]
            ctxTs = [sAtt.tile([128, SQ], FP16, name=f"ctxT{hl}")
                     for hl in range(HPC)]

            # ---- sweeps A/B: qrotT/krotT per head, k-streamed ----
            with tc.tile_pool(name="pQ", bufs=8, space="PSUM") as pQ:
                for hl in range(HPC):
                    psq = [pQ.tile([128, 512], F32, tag="acc",
                                   name=f"psq{hl}_{sc}", bufs=8)
                           for sc in range(4)]
                    psk = [pQ.tile([128, 512], F32, tag="acc",
                                   name=f"psk{hl}_{sc}", bufs=8)
                           for sc in range(4)]
                    for k in range(KT):
                        for sc in range(4):
                            rhs = hT_sb[:, k * SQ + sc * 512:
                                        k * SQ + (sc + 1) * 512]
                            nc.tensor.matmul(
                                psq[sc][:],
                                wqk_sb[:, k * 512 + hl * 256:
                                       k * 512 + hl * 256 + 128],
                                rhs, start=(k == 0), stop=(k == KT - 1))
                            nc.tensor.matmul(
                                psk[sc][:],
                                wqk_sb[:, k * 512 + hl * 256 + 128:
                                       k * 512 + hl * 256 + 256],
                                rhs, start=(k == 0), stop=(k == KT - 1))
                    for sc in range(4):
                        nc.vector.tensor_scalar_add(
                            qrotTs[hl][:, sc * 512:(sc + 1) * 512],
                            psq[sc][:], bqk_sb[:, 2 * hl:2 * hl + 1])
                        nc.vector.tensor_scalar_add(
                            krotTs[hl][:, sc * 512:(sc + 1) * 512],
                            psk[sc][:], bqk_sb[:, 2 * hl + 1:2 * hl + 2])

            # ---- sweep C: vrot both heads, chunk-major ----
            with tc.tile_pool(name="pV", bufs=4, space="PSUM") as pV:
                for cp in range(8):
                    vps = [pV.tile([128, 256], F32, tag="vp",
                                   name=f"vp{cp}_{i}", bufs=4)
                           for i in range(2)]
                    for k in range(KT):
                        for i in range(2):
                            c = cp * 2 + i
                            nc.tensor.matmul(
                                vps[i][:],
                                hT_sb[:, k * SQ + c * 128:
                                      k * SQ + (c + 1) * 128],
                                wv_sb[:, k * 256:(k + 1) * 256],
                                start=(k == 0), stop=(k == KT - 1))
                    for i in range(2):
                        c = cp * 2 + i
                        for hl in range(HPC):
                            nc.vector.tensor_tensor(
                                vrots[hl][:, c * 128:(c + 1) * 128],
                                vps[i][:, hl * 128:(hl + 1) * 128],
                                bvB_sb[:, hl * 128:(hl + 1) * 128],
                                mybir.AluOpType.add)

            # hT/wqk/wv dead -> release, then stream tsr into the space
            sQ.release()
            sT = tc.alloc_tile_pool(name="sT", bufs=1)
            tsrb_sb = sT.tile([128, KT * H], FP16, name="tsrb_sb")
            for kt in range(KT):
                nc.sync.dma_start(tsrb_sb[:, kt * H:(kt + 1) * H],
                                  tsr[kt * 128:(kt + 1) * 128, :])

            # ---- attention per head; A2A(h0) hides under h1 ----
            with (
                tc.tile_pool(name="sD", bufs=2) as sD,
                tc.tile_pool(name="pC", bufs=2, space="PSUM") as pC,
            ):
                for hl in range(HPC):
                    qrotT, krotT = qrotTs[hl], krotTs[hl]
                    vrot, ctxT_sb = vrots[hl], ctxTs[hl]

                    # two independent rb-chains interleaved per pair to
                    # keep the PE dense (p-state) within one head
                    for rbp in range(2):
                        rbs = [2 * rbp, 2 * rbp + 1]
                        ncbs = {rb: (4 * (rb + 1) if causal else KT)
                                for rb in rbs}
                        ctps = {rb: pC.tile([128, 512], F32, tag="ctp",
                                            name=f"ctp{hl}_{rb}")
                                for rb in rbs}
                        lps = {rb: pC.tile([1, 512], F32, tag="lp", bufs=2,
                                           name=f"lp{hl}_{rb}")
                               for rb in rbs}
                        maxcp = max(ncbs[rb] // 2 for rb in rbs)
                        for cp in range(maxcp):
                            for rb in rbs:
                                ncb = ncbs[rb]
                                if cp >= ncb // 2:
                                    continue
                                sp = pC.tile([128, 1024], F32, tag="sp",
                                             name=f"sp{hl}_{rb}_{cp}")
                                pT = sD.tile([128, 1024], BF16, tag="pT",
                                             bufs=6,
                                             name=f"pT{hl}_{rb}_{cp}")
                                for ch in range(2):
                                    cb = cp * 2 + ch
                                    sph = sp[:, ch * 512:(ch + 1) * 512]
                                    masked = causal and cb >= 4 * rb
                                    if masked:
                                        o = 384 - (cb * 128 - rb * 512)
                                        nc.tensor.matmul(
                                            sph, idb_sb[:],
                                            tbh_sb[:, o:o + 512],
                                            start=True, stop=False)
                                    nc.tensor.matmul(
                                        sph,
                                        krotT[:, cb * 128:(cb + 1) * 128],
                                        qrotT[:, rb * 512:(rb + 1) * 512],
                                        start=not masked, stop=True)
                                for ch in range(2):
                                    nc.scalar.activation(
                                        pT[:, ch * 512:(ch + 1) * 512],
                                        sp[:, ch * 512:(ch + 1) * 512],
                                        mybir.ActivationFunctionType.Exp)
                                for ch in range(2):
                                    cb = cp * 2 + ch
                                    nc.tensor.matmul(
                                        ctps[rb][:],
                                        vrot[:, cb * 128:(cb + 1) * 128],
                                        pT[:, ch * 512:(ch + 1) * 512],
                                        start=(cb == 0),
                                        stop=(cb == ncb - 1))
                                for ch in range(2):
                                    cb = cp * 2 + ch
                                    nc.tensor.matmul(
                                        lps[rb][:], onesb_sb[:, 0:1],
                                        pT[:, ch * 512:(ch + 1) * 512],
                                        start=(cb == 0),
                                        stop=(cb == ncb - 1))
                        for rb in rbs:
                            # normalize: lp -> SBUF, partition-broadcast,
                            # fast reciprocal (128 lanes), mult
                            lsb = sD.tile([1, 512], F32, tag="lsb",
                                          name=f"lsb{hl}_{rb}")
                            nc.vector.tensor_copy(lsb[:], lps[rb][:])
                            lball = sD.tile([128, 512], F32, tag="lball",
                                            name=f"lball{hl}_{rb}")
                            nc.gpsimd.partition_broadcast(lball[:], lsb[:])
                            linvb = sD.tile([128, 512], F32, tag="lb",
                                            name=f"linvb{hl}_{rb}")
                            nc.vector.reciprocal_approx_fast(linvb[:],
                                                             lball[:])
                            nc.vector.tensor_tensor(
                                ctxT_sb[:, rb * 512:(rb + 1) * 512],
                                ctps[rb][:],
                                linvb[:], mybir.AluOpType.mult)
                            # stage ctx columns (dest cores 2rb, 2rb+1)
                            nc.sync.dma_start(
                                ctx_ins[hl].rearrange(
                                    "b p s -> p b s")[:, 2 * rb:2 * rb + 2],
                                ctxT_sb[:, rb * 512:(rb + 1) * 512]
                                .rearrange("p (b s) -> p b s", b=2))
                    nc.gpsimd.collective_compute(
                        "AllToAll", mybir.AluOpType.bypass,
                        replica_groups=rg,
                        ins=[ctx_ins[hl][:].opt()],
                        outs=[ctx_as[hl][:].opt()])

            # ---- stage E: out = ctx_myblock @ tsr + dense_b, split so
            #      the h0 half of the contraction overlaps A2A(h1) ----
            with (
                tc.tile_pool(name="sE", bufs=2) as sE,
                tc.tile_pool(name="pE", bufs=8, space="PSUM") as pE,
            ):
                ctxa_sb = sE.tile([128, KT * SEQB], FP16, tag="ctxa", bufs=1)
                ops = [[pE.tile([128, 512], F32, tag="op", bufs=8,
                                name=f"op{mt}_{n}") for n in range(4)]
                       for mt in range(2)]
                # dense_b enters the accumulators via a K=1 f32r matmul
                for mt in range(2):
                    for n in range(4):
                        nc.tensor.matmul(
                            ops[mt][n][:], r(ones_sb[0:1, :]),
                            r(db_sb[:, n * 512:(n + 1) * 512]),
                            start=True, stop=False)
                # hl=0 half: rotate all 8 accumulators per b for ILP
                nc.sync.dma_start(
                    ctxa_sb[:].rearrange("p (b h s) -> h p b s",
                                         b=N_CORES, h=HPC)[0],
                    ctx_as[0][:].rearrange("b p s -> p b s"))
                for mt in range(2):
                    for b in range(N_CORES):
                        kt = b * HPC
                        for n in range(4):
                            nc.tensor.matmul(
                                ops[mt][n][:],
                                ctxa_sb[:, kt * SEQB + mt * 128:
                                        kt * SEQB + (mt + 1) * 128],
                                tsrb_sb[:, kt * H + n * 512:
                                        kt * H + (n + 1) * 512],
                                start=False, stop=False)
                # hl=1 half: chain-major so each accumulator finishes
                # staggered; its drain + out DMA overlaps the next chain
                nc.sync.dma_start(
                    ctxa_sb[:].rearrange("p (b h s) -> h p b s",
                                         b=N_CORES, h=HPC)[1],
                    ctx_as[1][:].rearrange("b p s -> p b s"))
                for mt in range(2):
                    for n in range(4):
                        for b in range(N_CORES):
                            kt = b * HPC + 1
                            nc.tensor.matmul(
                                ops[mt][n][:],
                                ctxa_sb[:, kt * SEQB + mt * 128:
                                        kt * SEQB + (mt + 1) * 128],
                                tsrb_sb[:, kt * H + n * 512:
                                        kt * H + (n + 1) * 512],
                                start=False, stop=(b == N_CORES - 1))
                        os_ = sE.tile([128, 512], F32, tag="os")
                        if (mt * 4 + n) % 2:
                            nc.vector.tensor_copy(os_[:], ops[mt][n][:])
                        else:
                            nc.scalar.copy(os_[:], ops[mt][n][:])
                        nc.sync.dma_start(
                            out[mt * 128:(mt + 1) * 128,
                                n * 512:(n + 1) * 512],
                            os_[:])
            sT.release()
            sAtt.release()
    nc.compile()
    return nc


_CAUSAL_MASK = None


def _is_causal(mask):
    global _CAUSAL_MASK
    m = np.asarray(mask).reshape(SQ, SQ)
    if _CAUSAL_MASK is None:
        _CAUSAL_MASK = np.triu(np.ones((SQ, SQ), dtype=bool), k=1)
    return np.array_equal(m, _CAUSAL_MASK)


def make_in_maps(inputs):
    hidden_states = np.asarray(inputs["hidden_states"], np.float32)
    qkv_w = np.asarray(inputs["qkv_w"], np.float32)
    qkv_b = np.asarray(inputs["qkv_b"], np.float32)
    svd_token = np.ascontiguousarray(np.asarray(inputs["svd_token"],
                                                np.float32))
    svd_qk = np.asarray(inputs["svd_qk"], np.float32)
    svd_vlin = np.asarray(inputs["svd_vlin"], np.float32)
    dense_w = np.asarray(inputs["dense_w"], np.float32)
    dense_b = np.asarray(inputs["dense_b"], np.float32)

    # host fold: mixed = X (S S^T Q^T) + b; then per-head q/k/v rotations
    # and the softmax scale (split sqrt into q and k) fold into W/b too.
    sq_scale = math.sqrt(SCALE)
    G = svd_token @ svd_token.T
    Wmix = G @ qkv_w.T                                    # [H, 3H]
    Wh = Wmix.reshape(H, NH, 3 * HD)
    bh = qkv_b.reshape(NH, 3 * HD)
    Wq = np.einsum("xhd,hde->xhe", Wh[:, :, :HD], svd_qk,
                   optimize=True) * sq_scale
    Wk = np.einsum("xhd,hde->xhe", Wh[:, :, HD:2 * HD], svd_qk,
                   optimize=True) * sq_scale
    Wv = np.einsum("xhd,hde->xhe", Wh[:, :, 2 * HD:], svd_vlin,
                   optimize=True)
    bq = np.einsum("hd,hde->he", bh[:, :HD], svd_qk, optimize=True) * sq_scale
    bk = np.einsum("hd,hde->he", bh[:, HD:2 * HD], svd_qk,
                   optimize=True) * sq_scale
    bv = np.einsum("hd,hde->he", bh[:, 2 * HD:], svd_vlin, optimize=True)

    tsr = np.matmul(svd_vlin.transpose(0, 2, 1), dense_w).reshape(H, H)
    tsr_b = np.ascontiguousarray(tsr).astype(np.float16)
    hTf = np.ascontiguousarray(
        hidden_states[:, 0, :].T).astype(np.float16)      # [H, SQ]
    dbB = np.ascontiguousarray(dense_b.reshape(1, H))

    in_maps = []
    for c in range(N_CORES):
        h0 = c * HPC
        wqk_c = np.empty((H, 4 * HD), np.float32)
        wqk_c[:, 0:128] = Wq[:, h0]
        wqk_c[:, 128:256] = Wk[:, h0]
        wqk_c[:, 256:384] = Wq[:, h0 + 1]
        wqk_c[:, 384:512] = Wk[:, h0 + 1]
        wv_c = np.concatenate([Wv[:, h0], Wv[:, h0 + 1]], axis=1)
        bqk_c = np.stack([bq[h0], bk[h0], bq[h0 + 1], bk[h0 + 1]], axis=1)
        bv_c = np.concatenate([bv[h0], bv[h0 + 1]])
        in_maps.append({
            "hT": hTf,
            "wqk": wqk_c.astype(np.float16),
            "wv": wv_c.astype(np.float16),
            "bqk": np.ascontiguousarray(bqk_c, dtype=np.float32),
            "bvB": np.ascontiguousarray(
                np.broadcast_to(bv_c[None, :], (128, 2 * HD)),
                dtype=np.float32),
            "tsr": tsr_b,
            "dbB": dbB,
        })
    return in_maps


def kernel(hidden_states, attention_mask, qkv_w, qkv_b, svd_token,
           svd_qk, svd_vlin, dense_w, dense_b):
    causal = _is_causal(attention_mask)
    if not causal:
        assert not np.asarray(attention_mask).any(), \
            "kernel supports causal or empty attention_mask"

    nc = build(causal=causal)
    in_maps = make_in_maps({
        "hidden_states": hidden_states, "qkv_w": qkv_w, "qkv_b": qkv_b,
        "svd_token": svd_token, "svd_qk": svd_qk, "svd_vlin": svd_vlin,
        "dense_w": dense_w, "dense_b": dense_b,
    })
    res = bass_utils.run_bass_kernel_spmd(
        nc, in_maps, core_ids=list(range(N_CORES)), trace=False)
    full = np.concatenate([res.results[c]["out"] for c in range(N_CORES)],
                          axis=0)
    return full.reshape(SQ, 1, H)


# revision 28
# speedup vs baseline: 1.4602x; 1.0025x over previous
"""Trainium2 8-core tensor-parallel sparse-attention kernel (Bass/Tile).

Reference (SQ=2048, B=1, H=2048, NH=16, HD=128):
    x = hidden[:,0,:] @ svd_token
    w = qkv_w @ svd_token;  mixed = x @ w.T + qkv_b
    per head h: q,k rotated by svd_qk[h], v by svd_vlin[h]
    scores = qr @ kr.T / sqrt(128) causal-masked, softmax
    ctx = probs @ vr;  tsr[h] = svd_vlin[h].T @ dense_w[h]
    out = ctx @ tsr + dense_b

Key identity (host fold): mixed = X @ (S S^T Q^T) + b, and the per-head
q/k/v rotations are linear, so the WHOLE projection collapses to
    qrot_h = X @ Wq_h + bq_h,  Wq_h = (S S^T Q^T)[:, qcols_h] @ svd_qk[h]
(same for k, v with svd_vlin), all folded in fp32 on the host. tsr is
also fully precomputed host-side. Device work per core drops to:
  Q:   qrotT/krotT/vrot for 2 heads = X-contraction only (6.4 GF)
  att: causal-blocked scores/exp/PV with band-preload masking
  E:   out_block = ctx @ tsr + dense_b after per-head A2A of ctx
No AllGathers of intermediates remain (only warmup + 2 per-head A2A).

Per-core pipeline (TP over heads, 2 heads/core):
  warmup AG first (CC stream init ~55us)
  hT streams on sync queue; wqk/wv/biases on gpsimd queue
  sweep A/B: qrotT,krotT per head  psum[128,512]x8, k-streamed
  sweep C:   vrot both heads, chunk-major psum[128,256], k inner
  tsr (8MB) loads on sync queue after hT's pool releases
  attention per head: two rb-chains interleaved; causal band preloaded
  into PSUM by an identity matmul; raw exp on [128,1024] tiles; P@V +
  ones row-sum; normalize via partition_broadcast + reciprocal
  A2A(h0) hides under h1's attention; stage E splits h0/h1 halves so
  the h0 half of the contraction overlaps A2A(h1); dense_b enters via
  a K=1 f32r matmul preload of the output accumulators.
Host only shards/folds inputs and concatenates the 8 output row-blocks.
"""
import math

import ml_dtypes
import numpy as np

import concourse.bass as bass
import concourse.mybir as mybir
import concourse.bacc as bacc
import concourse.tile as tile
from concourse import bass_utils

N_CORES = 8
SQ = 2048
H = 2048
NH = 16
HD = 128
HPC = NH // N_CORES          # heads per core = 2
SEQB = SQ // N_CORES         # seq block per core = 256
KT = H // 128                # 128-tiles over hidden = 16
F32 = mybir.dt.float32
F32R = mybir.dt.float32r
BF16 = mybir.dt.bfloat16
FP16 = mybir.dt.float16
SCALE = 1.0 / math.sqrt(HD)


def r(ap):
    return ap.bitcast(F32R)


def build(causal=True):
    nc = bacc.Bacc("TRN2", target_bir_lowering=False, debug=False,
                   num_devices=N_CORES)

    hT = nc.dram_tensor("hT", [H, SQ], FP16, kind="ExternalInput")
    wqk = nc.dram_tensor("wqk", [H, 4 * HD], FP16, kind="ExternalInput")
    wv = nc.dram_tensor("wv", [H, 2 * HD], FP16, kind="ExternalInput")
    bqk = nc.dram_tensor("bqk", [128, 4], F32, kind="ExternalInput")
    bvB = nc.dram_tensor("bvB", [128, 2 * HD], F32, kind="ExternalInput")
    tsr = nc.dram_tensor("tsr", [H, H], FP16, kind="ExternalInput")
    dbB = nc.dram_tensor("dbB", [1, H], F32, kind="ExternalInput")
    out = nc.dram_tensor("out", [SEQB, H], F32, kind="ExternalOutput")

    ones_dram = nc.inline_tensor(np.ones((128, 128), np.float32), name="ones_c")
    onesb_dram = nc.inline_tensor(np.ones((128, 128), ml_dtypes.bfloat16),
                                  name="onesb_c")
    idh_dram = nc.inline_tensor(np.eye(128, dtype=np.float16), name="idh_c")
    # additive causal mask band (-30000 above diagonal), preloaded
    # into PSUM via an identity matmul so masking never leaves the PE
    tbh_np = np.where(
        np.arange(128)[:, None] > np.arange(896)[None, :] - 384, -30000.0, 0.0
    ).astype(np.float16)
    tbh_dram = nc.inline_tensor(tbh_np, name="tbh_c")

    rg = [list(range(N_CORES))]

    with tile.TileContext(nc) as tc:
        with (
            nc.allow_low_precision(reason="f32r/bf16 for full-rate PE"),
            tc.tile_pool(name="pers", bufs=1) as pers,
            tc.tile_pool(name="dram", bufs=1, space="DRAM") as dram,
        ):
            # ---- warmup collective ASAP (CC stream init ~55us) ----
            warm_in = dram.tile([128, 128], F32)
            warm_out = dram.tile([N_CORES * 128, 128], F32,
                                 addr_space="Shared")
            nc.sync.dma_start(warm_in[:], ones_dram[:])
            nc.gpsimd.collective_compute(
                "AllGather", mybir.AluOpType.bypass, replica_groups=rg,
                ins=[warm_in[:].opt()], outs=[warm_out[:].opt()])

            # ---- persistent constants (gpsimd queue) ----
            ones_sb = pers.tile([128, 128], F32)
            onesb_sb = pers.tile([128, 128], BF16)
            tbh_sb = pers.tile([128, 896], FP16)
            idb_sb = pers.tile([128, 128], FP16)
            nc.gpsimd.dma_start(idb_sb[:], idh_dram[:])
            nc.gpsimd.dma_start(r(ones_sb[:]), r(ones_dram[:]))
            nc.gpsimd.dma_start(onesb_sb[:], onesb_dram[:])
            nc.gpsimd.dma_start(tbh_sb[:], tbh_dram[:])
            bqk_sb = pers.tile([128, 4], F32)
            nc.gpsimd.dma_start(bqk_sb[:], bqk[:])
            bvB_sb = pers.tile([128, 2 * HD], F32)
            nc.gpsimd.dma_start(r(bvB_sb[:]), r(bvB[:]))
            db_sb = pers.tile([1, H], F32)
            nc.gpsimd.dma_start(r(db_sb[:]), r(dbB[:]))

            ctx_ins = [dram.tile([N_CORES, HD, SEQB], FP16,
                                 name=f"ctxin{hl}") for hl in range(HPC)]
            ctx_as = [dram.tile([N_CORES, HD, SEQB], FP16,
                                name=f"ctxa{hl}") for hl in range(HPC)]

            # ---- stage Q inputs, all on the sync queue in consumption
            #      order; per-k/per-pair tiles so the first matmul only
            #      waits on its own slice, not the whole stream ----
            sQ = tc.alloc_tile_pool(name="sQ", bufs=1)
            wqk_t = [sQ.tile([128, 4 * HD], FP16, name=f"wqk{k}")
                     for k in range(KT)]
            wv_sb = sQ.tile([128, KT * 2 * HD], FP16, name="wv_sb")
            hT_t = [sQ.tile([128, 2 * SQ], FP16, name=f"hT{k2}")
                    for k2 in range(KT // 2)]
            hT_v = hT.rearrange("(k p) s -> p k s", p=128)
            nc.sync.dma_start(wqk_t[0][:], wqk[0:128, :])
            nc.sync.dma_start(
                hT_t[0][:].rearrange("p (k s) -> p k s", k=2), hT_v[:, 0:2])
            for k in range(1, KT):
                nc.sync.dma_start(wqk_t[k][:],
                                  wqk[k * 128:(k + 1) * 128, :])
            for k2 in range(1, KT // 2):
                nc.sync.dma_start(
                    hT_t[k2][:].rearrange("p (k s) -> p k s", k=2),
                    hT_v[:, k2 * 2:(k2 + 1) * 2])
            nc.sync.dma_start(
                wv_sb[:].rearrange("p (k c) -> p k c", k=KT),
                wv.rearrange("(k p) c -> p k c", p=128))

            def hTs(k, lo, hi):
                return hT_t[k // 2][:, (k % 2) * SQ + lo:(k % 2) * SQ + hi]

            # attention-lifetime SBUF tiles (right side: outlives sQ,
            # which must release in LIFO order on the left stack)
            sAtt = tc.alloc_tile_pool(name="sAtt", bufs=1, side="right")
            qrotTs = [sAtt.tile([128, SQ], FP16, name=f"qrotT{hl}")
                      for hl in range(HPC)]
            krotTs = [sAtt.tile([128, SQ], FP16, name=f"krotT{hl}")
                      for hl in range(HPC)]
            vrots = [sAtt.tile([128, SQ], BF16, name=f"vrot{hl}")
                     for hl in range(HPC)]
            ctxTs = [sAtt.tile([128, SQ], FP16, name=f"ctxT{hl}")
                     for hl in range(HPC)]

            # ---- sweeps A/B: qrotT/krotT per head, k-streamed ----
            with tc.tile_pool(name="pQ", bufs=8, space="PSUM") as pQ:
                for hl in range(HPC):
                    psq = [pQ.tile([128, 512], F32, tag="acc",
                                   name=f"psq{hl}_{sc}", bufs=8)
                           for sc in range(4)]
                    psk = [pQ.tile([128, 512], F32, tag="acc",
                                   name=f"psk{hl}_{sc}", bufs=8)
                           for sc in range(4)]
                    for k in range(KT):
                        for sc in range(4):
                            rhs = hTs(k, sc * 512, (sc + 1) * 512)
                            nc.tensor.matmul(
                                psq[sc][:],
                                wqk_t[k][:, hl * 256:hl * 256 + 128],
                                rhs, start=(k == 0), stop=(k == KT - 1))
                            nc.tensor.matmul(
                                psk[sc][:],
                                wqk_t[k][:, hl * 256 + 128:hl * 256 + 256],
                                rhs, start=(k == 0), stop=(k == KT - 1))
                    for sc in range(4):
                        nc.vector.tensor_scalar_add(
                            qrotTs[hl][:, sc * 512:(sc + 1) * 512],
                            psq[sc][:], bqk_sb[:, 2 * hl:2 * hl + 1])
                        nc.vector.tensor_scalar_add(
                            krotTs[hl][:, sc * 512:(sc + 1) * 512],
                            psk[sc][:], bqk_sb[:, 2 * hl + 1:2 * hl + 2])

            # ---- sweep C: vrot both heads, chunk-major ----
            with tc.tile_pool(name="pV", bufs=4, space="PSUM") as pV:
                for cp in range(8):
                    vps = [pV.tile([128, 256], F32, tag="vp",
                                   name=f"vp{cp}_{i}", bufs=4)
                           for i in range(2)]
                    for k in range(KT):
                        for i in range(2):
                            c = cp * 2 + i
                            nc.tensor.matmul(
                                vps[i][:],
                                hTs(k, c * 128, (c + 1) * 128),
                                wv_sb[:, k * 256:(k + 1) * 256],
                                start=(k == 0), stop=(k == KT - 1))
                    for i in range(2):
                        c = cp * 2 + i
                        for hl in range(HPC):
                            nc.vector.tensor_tensor(
                                vrots[hl][:, c * 128:(c + 1) * 128],
                                vps[i][:, hl * 128:(hl + 1) * 128],
                                bvB_sb[:, hl * 128:(hl + 1) * 128],
                                mybir.AluOpType.add)

            # hT/wqk/wv dead -> release, then stream tsr into the space
            sQ.release()
            sT = tc.alloc_tile_pool(name="sT", bufs=1)
            tsrb_sb = sT.tile([128, KT * H], FP16, name="tsrb_sb")
            for kt in range(KT):
                nc.sync.dma_start(tsrb_sb[:, kt * H:(kt + 1) * H],
                                  tsr[kt * 128:(kt + 1) * 128, :])

            # ---- attention per head; A2A(h0) hides under h1 ----
            with (
                tc.tile_pool(name="sD", bufs=2) as sD,
                tc.tile_pool(name="pC", bufs=2, space="PSUM") as pC,
            ):
                for hl in range(HPC):
                    qrotT, krotT = qrotTs[hl], krotTs[hl]
                    vrot, ctxT_sb = vrots[hl], ctxTs[hl]
                    # two independent rb-chains interleaved per pair to
                    # keep the PE dense (p-state) within one head
                    for rbp in range(2):
                        rbs = [2 * rbp, 2 * rbp + 1]
                        ncbs = {rb: (4 * (rb + 1) if causal else KT)
                                for rb in rbs}
                        ctps = {rb: pC.tile([128, 512], F32, tag="ctp",
                                            bufs=3, name=f"ctp{hl}_{rb}")
                                for rb in rbs}
                        lps = {rb: pC.tile([1, 512], F32, tag="lp", bufs=2,
                                           name=f"lp{hl}_{rb}")
                               for rb in rbs}
                        maxcp = max(ncbs[rb] // 2 for rb in rbs)
                        for cp in range(maxcp):
                            for rb in rbs:
                                ncb = ncbs[rb]
                                if cp >= ncb // 2:
                                    continue
                                pTs = []
                                for ch in range(2):
                                    cb = cp * 2 + ch
                                    sp = pC.tile([128, 512], F32, tag="sp",
                                                 bufs=3,
                                                 name=f"sp{hl}_{rb}_{cb}")
                                    pT = sD.tile([128, 512], BF16,
                                                 tag="pT", bufs=8,
                                                 name=f"pT{hl}_{rb}_{cb}")
                                    pTs.append(pT)
                                    masked = causal and cb >= 4 * rb
                                    if masked:
                                        o = 384 - (cb * 128 - rb * 512)
                                        nc.tensor.matmul(
                                            sp[:], idb_sb[:],
                                            tbh_sb[:, o:o + 512],
                                            start=True, stop=False)
                                    nc.tensor.matmul(
                                        sp[:],
                                        krotT[:, cb * 128:(cb + 1) * 128],
                                        qrotT[:, rb * 512:(rb + 1) * 512],
                                        start=not masked, stop=True)
                                    nc.scalar.activation(
                                        pT[:], sp[:],
                                        mybir.ActivationFunctionType.Exp)
                                for ch in range(2):
                                    cb = cp * 2 + ch
                                    nc.tensor.matmul(
                                        ctps[rb][:],
                                        vrot[:, cb * 128:(cb + 1) * 128],
                                        pTs[ch][:],
                                        start=(cb == 0),
                                        stop=(cb == ncb - 1))
                                    nc.tensor.matmul(
                                        lps[rb][:],
                                        onesb_sb[:, 0:1], pTs[ch][:],
                                        start=(cb == 0),
                                        stop=(cb == ncb - 1))
                        for rb in rbs:
                            # normalize: lp -> SBUF (gpsimd cannot read
                            # PSUM), partition-broadcast, fast reciprocal
                            lsb = sD.tile([1, 512], F32, tag="lsb",
                                          name=f"lsb{hl}_{rb}")
                            nc.vector.tensor_copy(lsb[:], lps[rb][:])
                            lball = sD.tile([128, 512], F32, tag="lball",
                                            name=f"lball{hl}_{rb}")
                            nc.gpsimd.partition_broadcast(lball[:], lsb[:])
                            linvb = sD.tile([128, 512], F32, tag="lb",
                                            name=f"linvb{hl}_{rb}")
                            nc.vector.reciprocal_approx_fast(linvb[:],
                                                             lball[:])
                            nc.vector.tensor_tensor(
                                ctxT_sb[:, rb * 512:(rb + 1) * 512],
                                ctps[rb][:],
                                linvb[:], mybir.AluOpType.mult)
                            # stage ctx columns (dest cores 2rb, 2rb+1)
                            nc.sync.dma_start(
                                ctx_ins[hl].rearrange(
                                    "b p s -> p b s")[:, 2 * rb:2 * rb + 2],
                                ctxT_sb[:, rb * 512:(rb + 1) * 512]
                                .rearrange("p (b s) -> p b s", b=2))
                    nc.gpsimd.collective_compute(
                        "AllToAll", mybir.AluOpType.bypass,
                        replica_groups=rg,
                        ins=[ctx_ins[hl][:].opt()],
                        outs=[ctx_as[hl][:].opt()])

            # ---- stage E: out = ctx_myblock @ tsr + dense_b, split so
            #      the h0 half of the contraction overlaps A2A(h1) ----
            with (
                tc.tile_pool(name="sE", bufs=2) as sE,
                tc.tile_pool(name="pE", bufs=8, space="PSUM") as pE,
            ):
                # per-(hl,b) ctx tiles: deps are tile-granular, so the
                # first E matmul only waits on its own 64KB slice
                ctxa_t = [[sE.tile([128, SEQB], FP16, bufs=1,
                                   name=f"cx{hl}_{b}")
                           for b in range(N_CORES)] for hl in range(HPC)]
                ops = [[pE.tile([128, 512], F32, tag="op", bufs=8,
                                name=f"op{mt}_{n}") for n in range(4)]
                       for mt in range(2)]
                # dense_b enters the accumulators via a K=1 f32r matmul
                for mt in range(2):
                    for n in range(4):
                        nc.tensor.matmul(
                            ops[mt][n][:], r(ones_sb[0:1, :]),
                            r(db_sb[:, n * 512:(n + 1) * 512]),
                            start=True, stop=False)
                # hl=0 half: rotate all 8 accumulators per b for ILP
                for b in range(N_CORES):
                    nc.sync.dma_start(ctxa_t[0][b][:], ctx_as[0][b])
                for mt in range(2):
                    for b in range(N_CORES):
                        kt = b * HPC
                        for n in range(4):
                            nc.tensor.matmul(
                                ops[mt][n][:],
                                ctxa_t[0][b][:, mt * 128:(mt + 1) * 128],
                                tsrb_sb[:, kt * H + n * 512:
                                        kt * H + (n + 1) * 512],
                                start=False, stop=False)
                # hl=1 half: chain-major so each accumulator finishes
                # staggered; its drain + out DMA overlaps the next chain
                for b in range(N_CORES):
                    nc.sync.dma_start(ctxa_t[1][b][:], ctx_as[1][b])
                for mt in range(2):
                    for n in range(4):
                        for b in range(N_CORES):
                            kt = b * HPC + 1
                            nc.tensor.matmul(
                                ops[mt][n][:],
                                ctxa_t[1][b][:, mt * 128:(mt + 1) * 128],
                                tsrb_sb[:, kt * H + n * 512:
                                        kt * H + (n + 1) * 512],
                                start=False, stop=(b == N_CORES - 1))
                        os_ = sE.tile([128, 512], F32, tag="os")
                        if (mt * 4 + n) % 2:
                            nc.vector.tensor_copy(os_[:], ops[mt][n][:])
                        else:
                            nc.scalar.copy(os_[:], ops[mt][n][:])
                        nc.sync.dma_start(
                            out[mt * 128:(mt + 1) * 128,
                                n * 512:(n + 1) * 512],
                            os_[:])
            sT.release()
            sAtt.release()
    nc.compile()
    return nc


_CAUSAL_MASK = None


def _is_causal(mask):
    global _CAUSAL_MASK
    m = np.asarray(mask).reshape(SQ, SQ)
    if _CAUSAL_MASK is None:
        _CAUSAL_MASK = np.triu(np.ones((SQ, SQ), dtype=bool), k=1)
    return np.array_equal(m, _CAUSAL_MASK)


def make_in_maps(inputs):
    hidden_states = np.asarray(inputs["hidden_states"], np.float32)
    qkv_w = np.asarray(inputs["qkv_w"], np.float32)
    qkv_b = np.asarray(inputs["qkv_b"], np.float32)
    svd_token = np.ascontiguousarray(np.asarray(inputs["svd_token"],
                                                np.float32))
    svd_qk = np.asarray(inputs["svd_qk"], np.float32)
    svd_vlin = np.asarray(inputs["svd_vlin"], np.float32)
    dense_w = np.asarray(inputs["dense_w"], np.float32)
    dense_b = np.asarray(inputs["dense_b"], np.float32)

    # host fold: mixed = X (S S^T Q^T) + b; then per-head q/k/v rotations
    # and the softmax scale (split sqrt into q and k) fold into W/b too.
    sq_scale = math.sqrt(SCALE)
    G = svd_token @ svd_token.T
    Wmix = G @ qkv_w.T                                    # [H, 3H]
    Wh = Wmix.reshape(H, NH, 3 * HD)
    bh = qkv_b.reshape(NH, 3 * HD)
    Wq = np.einsum("xhd,hde->xhe", Wh[:, :, :HD], svd_qk,
                   optimize=True) * sq_scale
    Wk = np.einsum("xhd,hde->xhe", Wh[:, :, HD:2 * HD], svd_qk,
                   optimize=True) * sq_scale
    Wv = np.einsum("xhd,hde->xhe", Wh[:, :, 2 * HD:], svd_vlin,
                   optimize=True)
    bq = np.einsum("hd,hde->he", bh[:, :HD], svd_qk, optimize=True) * sq_scale
    bk = np.einsum("hd,hde->he", bh[:, HD:2 * HD], svd_qk,
                   optimize=True) * sq_scale
    bv = np.einsum("hd,hde->he", bh[:, 2 * HD:], svd_vlin, optimize=True)

    tsr = np.matmul(svd_vlin.transpose(0, 2, 1), dense_w).reshape(H, H)
    tsr_b = np.ascontiguousarray(tsr).astype(np.float16)
    hTf = np.ascontiguousarray(
        hidden_states[:, 0, :].T).astype(np.float16)      # [H, SQ]
    dbB = np.ascontiguousarray(dense_b.reshape(1, H))

    in_maps = []
    for c in range(N_CORES):
        h0 = c * HPC
        wqk_c = np.empty((H, 4 * HD), np.float32)
        wqk_c[:, 0:128] = Wq[:, h0]
        wqk_c[:, 128:256] = Wk[:, h0]
        wqk_c[:, 256:384] = Wq[:, h0 + 1]
        wqk_c[:, 384:512] = Wk[:, h0 + 1]
        wv_c = np.concatenate([Wv[:, h0], Wv[:, h0 + 1]], axis=1)
        bqk_c = np.stack([bq[h0], bk[h0], bq[h0 + 1], bk[h0 + 1]], axis=1)
        bv_c = np.concatenate([bv[h0], bv[h0 + 1]])
        in_maps.append({
            "hT": hTf,
            "wqk": wqk_c.astype(np.float16),
            "wv": wv_c.astype(np.float16),
            "bqk": np.ascontiguousarray(bqk_c, dtype=np.float32),
            "bvB": np.ascontiguousarray(
                np.broadcast_to(bv_c[None, :], (128, 2 * HD)),
                dtype=np.float32),
            "tsr": tsr_b,
            "dbB": dbB,
        })
    return in_maps


def kernel(hidden_states, attention_mask, qkv_w, qkv_b, svd_token,
           svd_qk, svd_vlin, dense_w, dense_b):
    causal = _is_causal(attention_mask)
    if not causal:
        assert not np.asarray(attention_mask).any(), \
            "kernel supports causal or empty attention_mask"

    nc = build(causal=causal)
    in_maps = make_in_maps({
        "hidden_states": hidden_states, "qkv_w": qkv_w, "qkv_b": qkv_b,
        "svd_token": svd_token, "svd_qk": svd_qk, "svd_vlin": svd_vlin,
        "dense_w": dense_w, "dense_b": dense_b,
    })
    res = bass_utils.run_bass_kernel_spmd(
        nc, in_maps, core_ids=list(range(N_CORES)), trace=False)
    full = np.concatenate([res.results[c]["out"] for c in range(N_CORES)],
                          axis=0)
    return full.reshape(SQ, 1, H)


# revision 33
# speedup vs baseline: 1.4716x; 1.0078x over previous
"""Trainium2 8-core tensor-parallel sparse-attention kernel (Bass/Tile).

Reference (SQ=2048, B=1, H=2048, NH=16, HD=128):
    x = hidden[:,0,:] @ svd_token
    w = qkv_w @ svd_token;  mixed = x @ w.T + qkv_b
    per head h: q,k rotated by svd_qk[h], v by svd_vlin[h]
    scores = qr @ kr.T / sqrt(128) causal-masked, softmax
    ctx = probs @ vr;  tsr[h] = svd_vlin[h].T @ dense_w[h]
    out = ctx @ tsr + dense_b

Key identity (host fold): mixed = X @ (S S^T Q^T) + b, and the per-head
q/k/v rotations are linear, so the WHOLE projection collapses to
    qrot_h = X @ Wq_h + bq_h,  Wq_h = (S S^T Q^T)[:, qcols_h] @ svd_qk[h]
(same for k, v with svd_vlin), all folded in fp32 on the host. tsr is
also fully precomputed host-side. Device work per core drops to:
  Q:   qrotT/krotT/vrot for 2 heads = X-contraction only (6.4 GF)
  att: causal-blocked scores/exp/PV with band-preload masking
  E:   out_block = ctx @ tsr + dense_b after per-head A2A of ctx
No AllGathers of intermediates remain (only warmup + 2 per-head A2A).

Per-core pipeline (TP over heads, 2 heads/core):
  warmup AG first (CC stream init ~55us)
  hT streams on sync queue; wqk/wv/biases on gpsimd queue
  sweep A/B: qrotT,krotT per head  psum[128,512]x8, k-streamed
  sweep C:   vrot both heads, chunk-major psum[128,256], k inner
  tsr (8MB) loads on sync queue after hT's pool releases
  attention per head: two rb-chains interleaved; causal band preloaded
  into PSUM by an identity matmul; raw exp on [128,1024] tiles; P@V +
  ones row-sum; normalize via partition_broadcast + reciprocal
  A2A(h0) hides under h1's attention; stage E splits h0/h1 halves so
  the h0 half of the contraction overlaps A2A(h1); dense_b enters via
  a K=1 f32r matmul preload of the output accumulators.
Host only shards/folds inputs and concatenates the 8 output row-blocks.
"""
import math

import ml_dtypes
import numpy as np

import concourse.bass as bass
import concourse.mybir as mybir
import concourse.bacc as bacc
import concourse.tile as tile
from concourse import bass_utils

N_CORES = 8
SQ = 2048
H = 2048
NH = 16
HD = 128
HPC = NH // N_CORES          # heads per core = 2
SEQB = SQ // N_CORES         # seq block per core = 256
KT = H // 128                # 128-tiles over hidden = 16
F32 = mybir.dt.float32
F32R = mybir.dt.float32r
BF16 = mybir.dt.bfloat16
FP16 = mybir.dt.float16
SCALE = 1.0 / math.sqrt(HD)


def r(ap):
    return ap.bitcast(F32R)


def build(causal=True):
    nc = bacc.Bacc("TRN2", target_bir_lowering=False, debug=False,
                   num_devices=N_CORES)

    hT = nc.dram_tensor("hT", [H, SQ], FP16, kind="ExternalInput")
    wqk = nc.dram_tensor("wqk", [H, 4 * HD], FP16, kind="ExternalInput")
    wv = nc.dram_tensor("wv", [H, 2 * HD], FP16, kind="ExternalInput")
    bqk = nc.dram_tensor("bqk", [128, 4], F32, kind="ExternalInput")
    bvB = nc.dram_tensor("bvB", [128, 2 * HD], F32, kind="ExternalInput")
    tsr = nc.dram_tensor("tsr", [H, H], FP16, kind="ExternalInput")
    dbB = nc.dram_tensor("dbB", [1, H], F32, kind="ExternalInput")
    out = nc.dram_tensor("out", [SEQB, H], F32, kind="ExternalOutput")

    ones_dram = nc.inline_tensor(np.ones((128, 128), np.float32), name="ones_c")
    onesb_dram = nc.inline_tensor(np.ones((128, 128), ml_dtypes.bfloat16),
                                  name="onesb_c")
    idh_dram = nc.inline_tensor(np.eye(128, dtype=np.float16), name="idh_c")
    # additive causal mask band (-30000 above diagonal), preloaded
    # into PSUM via an identity matmul so masking never leaves the PE
    tbh_np = np.where(
        np.arange(128)[:, None] > np.arange(896)[None, :] - 384, -30000.0, 0.0
    ).astype(np.float16)
    tbh_dram = nc.inline_tensor(tbh_np, name="tbh_c")

    rg = [list(range(N_CORES))]

    with tile.TileContext(nc) as tc:
        with (
            nc.allow_low_precision(reason="f32r/bf16 for full-rate PE"),
            tc.tile_pool(name="pers", bufs=1) as pers,
            tc.tile_pool(name="dram", bufs=1, space="DRAM") as dram,
        ):
            # ---- warmup collective ASAP (CC stream init ~55us) ----
            warm_in = dram.tile([128, 128], F32)
            warm_out = dram.tile([N_CORES * 128, 128], F32,
                                 addr_space="Shared")
            nc.sync.dma_start(warm_in[:], ones_dram[:])
            nc.gpsimd.collective_compute(
                "AllGather", mybir.AluOpType.bypass, replica_groups=rg,
                ins=[warm_in[:].opt()], outs=[warm_out[:].opt()])

            # ---- persistent constants (gpsimd queue) ----
            ones_sb = pers.tile([128, 128], F32)
            onesb_sb = pers.tile([128, 128], BF16)
            tbh_sb = pers.tile([128, 896], FP16)
            idb_sb = pers.tile([128, 128], FP16)
            nc.gpsimd.dma_start(idb_sb[:], idh_dram[:])
            nc.gpsimd.dma_start(r(ones_sb[:]), r(ones_dram[:]))
            nc.gpsimd.dma_start(onesb_sb[:], onesb_dram[:])
            nc.gpsimd.dma_start(tbh_sb[:], tbh_dram[:])
            bqk_sb = pers.tile([128, 4], F32)
            nc.gpsimd.dma_start(bqk_sb[:], bqk[:])
            bvB_sb = pers.tile([128, 2 * HD], F32)
            nc.gpsimd.dma_start(r(bvB_sb[:]), r(bvB[:]))
            db_sb = pers.tile([1, H], F32)
            nc.gpsimd.dma_start(r(db_sb[:]), r(dbB[:]))

            ctx_ins = [dram.tile([N_CORES, HD, SEQB], FP16,
                                 name=f"ctxin{hl}") for hl in range(HPC)]
            ctx_as = [dram.tile([N_CORES, HD, SEQB], FP16,
                                name=f"ctxa{hl}") for hl in range(HPC)]

            # ---- stage Q inputs, all on the sync queue in consumption
            #      order; per-k/per-pair tiles so the first matmul only
            #      waits on its own slice, not the whole stream ----
            sQ = tc.alloc_tile_pool(name="sQ", bufs=1)
            wqk_t = [sQ.tile([128, 4 * HD], FP16, name=f"wqk{k}")
                     for k in range(KT)]
            wv_sb = sQ.tile([128, KT * 2 * HD], FP16, name="wv_sb")
            hT_t = [sQ.tile([128, 2 * SQ], FP16, name=f"hT{k2}")
                    for k2 in range(KT // 2)]
            hT_v = hT.rearrange("(k p) s -> p k s", p=128)
            # first slices from the (idle) scalar queue; rest interleaved
            # on the sync ring in consumption order
            nc.scalar.dma_start(wqk_t[0][:], wqk[0:128, :])
            nc.scalar.dma_start(
                hT_t[0][:].rearrange("p (k s) -> p k s", k=2), hT_v[:, 0:2])
            for k2 in range(1, KT // 2):
                nc.sync.dma_start(
                    hT_t[k2][:].rearrange("p (k s) -> p k s", k=2),
                    hT_v[:, k2 * 2:(k2 + 1) * 2])
                for k in (k2 * 2 - 1, k2 * 2):
                    nc.sync.dma_start(wqk_t[k][:],
                                      wqk[k * 128:(k + 1) * 128, :])
            nc.sync.dma_start(wqk_t[KT - 1][:],
                              wqk[(KT - 1) * 128:KT * 128, :])
            nc.sync.dma_start(
                wv_sb[:].rearrange("p (k c) -> p k c", k=KT),
                wv.rearrange("(k p) c -> p k c", p=128))

            def hTs(k, lo, hi):
                return hT_t[k // 2][:, (k % 2) * SQ + lo:(k % 2) * SQ + hi]

            # attention-lifetime SBUF tiles (right side: outlives sQ,
            # which must release in LIFO order on the left stack)
            sAtt = tc.alloc_tile_pool(name="sAtt", bufs=1, side="right")
            qrotTs = [sAtt.tile([128, SQ], FP16, name=f"qrotT{hl}")
                      for hl in range(HPC)]
            krotTs = [sAtt.tile([128, SQ], FP16, name=f"krotT{hl}")
                      for hl in range(HPC)]
            vrots = [sAtt.tile([128, SQ], BF16, name=f"vrot{hl}")
                     for hl in range(HPC)]
            ctxTs = [sAtt.tile([128, SQ], FP16, name=f"ctxT{hl}")
                     for hl in range(HPC)]

            # ---- sweeps A/B: qrotT/krotT per head, k-streamed ----
            with tc.tile_pool(name="pQ", bufs=8, space="PSUM") as pQ:
                for hl in range(HPC):
                    psq = [pQ.tile([128, 512], F32, tag="acc",
                                   name=f"psq{hl}_{sc}", bufs=8)
                           for sc in range(4)]
                    psk = [pQ.tile([128, 512], F32, tag="acc",
                                   name=f"psk{hl}_{sc}", bufs=8)
                           for sc in range(4)]
                    for k in range(KT):
                        for sc in range(4):
                            rhs = hTs(k, sc * 512, (sc + 1) * 512)
                            nc.tensor.matmul(
                                psq[sc][:],
                                wqk_t[k][:, hl * 256:hl * 256 + 128],
                                rhs, start=(k == 0), stop=(k == KT - 1))
                            nc.tensor.matmul(
                                psk[sc][:],
                                wqk_t[k][:, hl * 256 + 128:hl * 256 + 256],
                                rhs, start=(k == 0), stop=(k == KT - 1))
                    for sc in range(4):
                        nc.vector.tensor_scalar_add(
                            qrotTs[hl][:, sc * 512:(sc + 1) * 512],
                            psq[sc][:], bqk_sb[:, 2 * hl:2 * hl + 1])
                        nc.vector.tensor_scalar_add(
                            krotTs[hl][:, sc * 512:(sc + 1) * 512],
                            psk[sc][:], bqk_sb[:, 2 * hl + 1:2 * hl + 2])

            # ---- sweep C: vrot both heads, chunk-major ----
            with tc.tile_pool(name="pV", bufs=4, space="PSUM") as pV:
                for cp in range(8):
                    vps = [pV.tile([128, 256], F32, tag="vp",
                                   name=f"vp{cp}_{i}", bufs=4)
                           for i in range(2)]
                    for k in range(KT):
                        for i in range(2):
                            c = cp * 2 + i
                            nc.tensor.matmul(
                                vps[i][:],
                                hTs(k, c * 128, (c + 1) * 128),
                                wv_sb[:, k * 256:(k + 1) * 256],
                                start=(k == 0), stop=(k == KT - 1))
                    for i in range(2):
                        c = cp * 2 + i
                        for hl in range(HPC):
                            nc.vector.tensor_tensor(
                                vrots[hl][:, c * 128:(c + 1) * 128],
                                vps[i][:, hl * 128:(hl + 1) * 128],
                                bvB_sb[:, hl * 128:(hl + 1) * 128],
                                mybir.AluOpType.add)

            # hT/wqk/wv dead -> release, then stream tsr into the space
            sQ.release()
            sT = tc.alloc_tile_pool(name="sT", bufs=1)
            tsrb_sb = sT.tile([128, KT * H], FP16, name="tsrb_sb")
            for kt in range(KT):
                nc.sync.dma_start(tsrb_sb[:, kt * H:(kt + 1) * H],
                                  tsr[kt * 128:(kt + 1) * 128, :])

            # ---- attention per head; A2A(h0) hides under h1 ----
            with (
                tc.tile_pool(name="sD", bufs=2) as sD,
                tc.tile_pool(name="pC", bufs=2, space="PSUM") as pC,
            ):
                for hl in range(HPC):
                    qrotT, krotT = qrotTs[hl], krotTs[hl]
                    vrot, ctxT_sb = vrots[hl], ctxTs[hl]
                    # two independent rb-chains interleaved per pair to
                    # keep the PE dense (p-state) within one head
                    for rbp in range(2):
                        rbs = [2 * rbp, 2 * rbp + 1]
                        ncbs = {rb: (4 * (rb + 1) if causal else KT)
                                for rb in rbs}
                        ctps = {rb: pC.tile([128, 512], F32, tag="ctp",
                                            bufs=3, name=f"ctp{hl}_{rb}")
                                for rb in rbs}
                        lps = {rb: pC.tile([1, 512], F32, tag="lp", bufs=2,
                                           name=f"lp{hl}_{rb}")
                               for rb in rbs}
                        maxcp = max(ncbs[rb] // 2 for rb in rbs)
                        for cp in range(maxcp):
                            for rb in rbs:
                                ncb = ncbs[rb]
                                if cp >= ncb // 2:
                                    continue
                                pTs = []
                                for ch in range(2):
                                    cb = cp * 2 + ch
                                    sp = pC.tile([128, 512], F32, tag="sp",
                                                 bufs=3,
                                                 name=f"sp{hl}_{rb}_{cb}")
                                    pT = sD.tile([128, 512], BF16,
                                                 tag="pT", bufs=8,
                                                 name=f"pT{hl}_{rb}_{cb}")
                                    pTs.append(pT)
                                    masked = causal and cb >= 4 * rb
                                    if masked:
                                        o = 384 - (cb * 128 - rb * 512)
                                        nc.tensor.matmul(
                                            sp[:], idb_sb[:],
                                            tbh_sb[:, o:o + 512],
                                            start=True, stop=False)
                                    nc.tensor.matmul(
                                        sp[:],
                                        krotT[:, cb * 128:(cb + 1) * 128],
                                        qrotT[:, rb * 512:(rb + 1) * 512],
                                        start=not masked, stop=True)
                                    nc.scalar.activation(
                                        pT[:], sp[:],
                                        mybir.ActivationFunctionType.Exp)
                                for ch in range(2):
                                    cb = cp * 2 + ch
                                    nc.tensor.matmul(
                                        ctps[rb][:],
                                        vrot[:, cb * 128:(cb + 1) * 128],
                                        pTs[ch][:],
                                        start=(cb == 0),
                                        stop=(cb == ncb - 1))
                                    nc.tensor.matmul(
                                        lps[rb][:],
                                        onesb_sb[:, 0:1], pTs[ch][:],
                                        start=(cb == 0),
                                        stop=(cb == ncb - 1))
                        for rb in rbs:
                            # normalize: lp -> SBUF row, PE K=1 matmul
                            # broadcast (reuses the lp PSUM bufs), fast
                            # reciprocal, column-scale
                            lsb = sD.tile([1, 512], F32R, tag="lsb",
                                          name=f"lsb{hl}_{rb}")
                            nc.vector.tensor_copy(lsb[:], lps[rb][:])
                            lball = pC.tile([128, 512], F32, tag="lp",
                                            bufs=2, name=f"lball{hl}_{rb}")
                            nc.tensor.matmul(lball[:], r(ones_sb[0:1, :]),
                                             lsb[:],
                                             start=True, stop=True)
                            linvb = sD.tile([128, 512], F32, tag="lb",
                                            name=f"linvb{hl}_{rb}")
                            nc.vector.reciprocal_approx_fast(linvb[:],
                                                             lball[:])
                            nc.vector.tensor_tensor(
                                ctxT_sb[:, rb * 512:(rb + 1) * 512],
                                ctps[rb][:],
                                linvb[:], mybir.AluOpType.mult)
                            # stage ctx columns (dest cores 2rb, 2rb+1)
                            nc.sync.dma_start(
                                ctx_ins[hl].rearrange(
                                    "b p s -> p b s")[:, 2 * rb:2 * rb + 2],
                                ctxT_sb[:, rb * 512:(rb + 1) * 512]
                                .rearrange("p (b s) -> p b s", b=2))
                    nc.gpsimd.collective_compute(
                        "AllToAll", mybir.AluOpType.bypass,
                        replica_groups=rg,
                        ins=[ctx_ins[hl][:].opt()],
                        outs=[ctx_as[hl][:].opt()])

            # ---- stage E: out = ctx_myblock @ tsr + dense_b, split so
            #      the h0 half of the contraction overlaps A2A(h1) ----
            with (
                tc.tile_pool(name="sE", bufs=2) as sE,
                tc.tile_pool(name="pE", bufs=8, space="PSUM") as pE,
            ):
                # per-(hl,b) ctx tiles: deps are tile-granular, so the
                # first E matmul only waits on its own 64KB slice
                ctxa_t = [[sE.tile([128, SEQB], FP16, bufs=1,
                                   name=f"cx{hl}_{b}")
                           for b in range(N_CORES)] for hl in range(HPC)]
                ops = [[pE.tile([128, 512], F32, tag="op", bufs=8,
                                name=f"op{mt}_{n}") for n in range(4)]
                       for mt in range(2)]
                # dense_b enters the accumulators via a K=1 f32r matmul
                for mt in range(2):
                    for n in range(4):
                        nc.tensor.matmul(
                            ops[mt][n][:], r(ones_sb[0:1, :]),
                            r(db_sb[:, n * 512:(n + 1) * 512]),
                            start=True, stop=False)
                # hl=0 half: rotate all 8 accumulators per b for ILP
                # (ctxa loads ride the idle scalar queue so they don't
                # sit behind ctx staging on the sync ring)
                for b in range(N_CORES):
                    nc.scalar.dma_start(ctxa_t[0][b][:], ctx_as[0][b])
                for mt in range(2):
                    for b in range(N_CORES):
                        kt = b * HPC
                        for n in range(4):
                            nc.tensor.matmul(
                                ops[mt][n][:],
                                ctxa_t[0][b][:, mt * 128:(mt + 1) * 128],
                                tsrb_sb[:, kt * H + n * 512:
                                        kt * H + (n + 1) * 512],
                                start=False, stop=False)
                # hl=1 half: chain-major so each accumulator finishes
                # staggered; its drain + out DMA overlaps the next chain
                for b in range(N_CORES):
                    nc.scalar.dma_start(ctxa_t[1][b][:], ctx_as[1][b])
                for mt in range(2):
                    for n in range(4):
                        for b in range(N_CORES):
                            kt = b * HPC + 1
                            nc.tensor.matmul(
                                ops[mt][n][:],
                                ctxa_t[1][b][:, mt * 128:(mt + 1) * 128],
                                tsrb_sb[:, kt * H + n * 512:
                                        kt * H + (n + 1) * 512],
                                start=False, stop=(b == N_CORES - 1))
                        os_ = sE.tile([128, 512], F32, tag="os")
                        if (mt * 4 + n) % 2:
                            nc.vector.tensor_copy(os_[:], ops[mt][n][:])
                        else:
                            nc.scalar.copy(os_[:], ops[mt][n][:])
                        nc.sync.dma_start(
                            out[mt * 128:(mt + 1) * 128,
                                n * 512:(n + 1) * 512],
                            os_[:])
            sT.release()
            sAtt.release()
    nc.compile()
    return nc


_CAUSAL_MASK = None


def _is_causal(mask):
    global _CAUSAL_MASK
    m = np.asarray(mask).reshape(SQ, SQ)
    if _CAUSAL_MASK is None:
        _CAUSAL_MASK = np.triu(np.ones((SQ, SQ), dtype=bool), k=1)
    return np.array_equal(m, _CAUSAL_MASK)


def make_in_maps(inputs):
    hidden_states = np.asarray(inputs["hidden_states"], np.float32)
    qkv_w = np.asarray(inputs["qkv_w"], np.float32)
    qkv_b = np.asarray(inputs["qkv_b"], np.float32)
    svd_token = np.ascontiguousarray(np.asarray(inputs["svd_token"],
                                                np.float32))
    svd_qk = np.asarray(inputs["svd_qk"], np.float32)
    svd_vlin = np.asarray(inputs["svd_vlin"], np.float32)
    dense_w = np.asarray(inputs["dense_w"], np.float32)
    dense_b = np.asarray(inputs["dense_b"], np.float32)

    # host fold: mixed = X (S S^T Q^T) + b; then per-head q/k/v rotations
    # and the softmax scale (split sqrt into q and k) fold into W/b too.
    sq_scale = math.sqrt(SCALE)
    G = svd_token @ svd_token.T
    Wmix = G @ qkv_w.T                                    # [H, 3H]
    Wh = Wmix.reshape(H, NH, 3 * HD)
    bh = qkv_b.reshape(NH, 3 * HD)
    Wq = np.einsum("xhd,hde->xhe", Wh[:, :, :HD], svd_qk,
                   optimize=True) * sq_scale
    Wk = np.einsum("xhd,hde->xhe", Wh[:, :, HD:2 * HD], svd_qk,
                   optimize=True) * sq_scale
    Wv = np.einsum("xhd,hde->xhe", Wh[:, :, 2 * HD:], svd_vlin,
                   optimize=True)
    bq = np.einsum("hd,hde->he", bh[:, :HD], svd_qk, optimize=True) * sq_scale
    bk = np.einsum("hd,hde->he", bh[:, HD:2 * HD], svd_qk,
                   optimize=True) * sq_scale
    bv = np.einsum("hd,hde->he", bh[:, 2 * HD:], svd_vlin, optimize=True)

    tsr = np.matmul(svd_vlin.transpose(0, 2, 1), dense_w).reshape(H, H)
    tsr_b = np.ascontiguousarray(tsr).astype(np.float16)
    hTf = np.ascontiguousarray(
        hidden_states[:, 0, :].T).astype(np.float16)      # [H, SQ]
    dbB = np.ascontiguousarray(dense_b.reshape(1, H))

    in_maps = []
    for c in range(N_CORES):
        h0 = c * HPC
        wqk_c = np.empty((H, 4 * HD), np.float32)
        wqk_c[:, 0:128] = Wq[:, h0]
        wqk_c[:, 128:256] = Wk[:, h0]
        wqk_c[:, 256:384] = Wq[:, h0 + 1]
        wqk_c[:, 384:512] = Wk[:, h0 + 1]
        wv_c = np.concatenate([Wv[:, h0], Wv[:, h0 + 1]], axis=1)
        bqk_c = np.stack([bq[h0], bk[h0], bq[h0 + 1], bk[h0 + 1]], axis=1)
        bv_c = np.concatenate([bv[h0], bv[h0 + 1]])
        in_maps.append({
            "hT": hTf,
            "wqk": wqk_c.astype(np.float16),
            "wv": wv_c.astype(np.float16),
            "bqk": np.ascontiguousarray(bqk_c, dtype=np.float32),
            "bvB": np.ascontiguousarray(
                np.broadcast_to(bv_c[None, :], (128, 2 * HD)),
                dtype=np.float32),
            "tsr": tsr_b,
            "dbB": dbB,
        })
    return in_maps


def kernel(hidden_states, attention_mask, qkv_w, qkv_b, svd_token,
           svd_qk, svd_vlin, dense_w, dense_b):
    causal = _is_causal(attention_mask)
    if not causal:
        assert not np.asarray(attention_mask).any(), \
            "kernel supports causal or empty attention_mask"

    nc = build(causal=causal)
    in_maps = make_in_maps({
        "hidden_states": hidden_states, "qkv_w": qkv_w, "qkv_b": qkv_b,
        "svd_token": svd_token, "svd_qk": svd_qk, "svd_vlin": svd_vlin,
        "dense_w": dense_w, "dense_b": dense_b,
    })
    res = bass_utils.run_bass_kernel_spmd(
        nc, in_maps, core_ids=list(range(N_CORES)), trace=False)
    full = np.concatenate([res.results[c]["out"] for c in range(N_CORES)],
                          axis=0)
    return full.reshape(SQ, 1, H)
